# revision 1
# baseline (speedup 1.0000x reference)
# Trainium2 Bass kernel for nn_AttentionalPropagation (B=2, D=256, N=M=4096, H=4).
#
# Sharding: 8 cores; each batch (B=2) owns 4 cores; each core computes a
# 1024-column sequence shard of the output end-to-end (q/scores/softmax/attn/
# message/MLP). k,v are computed redundantly per core from the full `source`
# of its batch. The only cross-core communication is an AllReduce of the
# InstanceNorm partial (sum, sumsq) statistics within each 4-core batch group.
#
# Per-head layout trick: conv weights' output channels are permuted host-side
# so that head channels are contiguous on SBUF partitions (head h lives at
# partitions 64*(h%2) of channel-chunk h//2); this lets per-head matmuls run
# directly off partition-aligned slices (PE row/col tiling).
#
# Softmax: scores are built transposed ([m, n] with m on partitions), exp'd on
# ScalarE (scale=1/8 folded in, no max-subtraction needed: |s/8| < ~5), and the
# softmax denominator is obtained for free by augmenting v^T with a ones
# column in the attn matmul (out row 64 = sum_m exp). Normalization is a
# per-column reciprocal broadcast multiply.

import numpy as np

import concourse.bass as bass  # noqa: F401  (bass types used via tile/bacc)
import concourse.tile as tile
import concourse.mybir as mybir
from concourse import bacc
from concourse import bass_utils

B, D, N = 2, 256, 4096
H, DH = 4, 64
NS = N // 4           # sequence shard per core
NCORES = 8
EPS = 1e-5

FP = mybir.dt.float32
BF = mybir.dt.bfloat16
F8 = mybir.dt.float8e4
AX = mybir.AxisListType
OP = mybir.AluOpType
AF = mybir.ActivationFunctionType

# j-groups for the scores->exp pipeline. Each group's PSUM tile holds BOTH
# heads of the current pair (2 x glen x 512 fp32): glen=2 -> 4 banks,
# glen=1 -> 2 banks; the two tiles double-buffer within 6 free banks and the
# two per-head attn accumulators take the other 2.
_JGROUPS = []
_j = 0
while _j < 32:
    g = 2 if (len(_JGROUPS) % 2 == 0 and _j + 2 <= 32) else 1
    _JGROUPS.append((_j, g))
    _j += g

import os
_STAGE = os.environ.get("KSTAGE", "full")  # debug bisection: proj|attn|nocc|full


def _emit(nc, tc, io, es):
    xs, src = io["xs"], io["src"]
    out = io["out"]

    wpool = es.enter_context(tc.tile_pool(name="weights", bufs=1))
    apool = es.enter_context(tc.tile_pool(name="acts", bufs=1))

    # ---------- weight / bias loads ----------
    wq_sb = wpool.tile([128, 2, D], FP)
    nc.sync.dma_start(out=wq_sb[:], in_=io["wqT"].rearrange("(c p) o -> p c o", p=128))
    wk_sb = wpool.tile([128, 2, D], FP)
    nc.sync.dma_start(out=wk_sb[:], in_=io["wkT"].rearrange("(c p) o -> p c o", p=128))
    wv_sb = wpool.tile([128, 2, D], FP)
    nc.sync.dma_start(out=wv_sb[:], in_=io["wvT"].rearrange("(c p) o -> p c o", p=128))
    wm_sb = wpool.tile([128, 2, D], BF)
    nc.gpsimd.dma_start(out=wm_sb[:], in_=io["wmT"].rearrange("(c p) o -> p c o", p=128))
    w1x_sb = wpool.tile([128, 2, 2 * D], FP)
    nc.sync.dma_start(out=w1x_sb[:], in_=io["w1xT"].rearrange("(c p) o -> p c o", p=128))
    w1m_sb = wpool.tile([128, 2, 2 * D], BF)
    nc.gpsimd.dma_start(out=w1m_sb[:], in_=io["w1mT"].rearrange("(c p) o -> p c o", p=128))
    w2_sb = wpool.tile([128, 4, D], BF)
    nc.gpsimd.dma_start(out=w2_sb[:], in_=io["w2T"].rearrange("(c p) o -> p c o", p=128))

    bq_sb = wpool.tile([128, 2], FP)
    nc.sync.dma_start(out=bq_sb[:], in_=io["bq"][:])
    bk_sb = wpool.tile([128, 2], FP)
    nc.sync.dma_start(out=bk_sb[:], in_=io["bk"][:])
    bm_sb = wpool.tile([128, 2], FP)
    nc.sync.dma_start(out=bm_sb[:], in_=io["bm"][:])
    b1_sb = wpool.tile([128, 4], FP)
    nc.sync.dma_start(out=b1_sb[:], in_=io["b1"][:])
    b2_sb = wpool.tile([128, 2], FP)
    nc.sync.dma_start(out=b2_sb[:], in_=io["b2"][:])
    bv_sb = wpool.tile([1, D], FP)
    nc.sync.dma_start(out=bv_sb[:], in_=io["bv"][:])
    bvb_sb = wpool.tile([128, D], FP)
    nc.gpsimd.partition_broadcast(bvb_sb[:], bv_sb[:])

    xs_sb = apool.tile([128, 2, NS], FP)
    nc.sync.dma_start(out=xs_sb[:], in_=xs.rearrange("(c p) n -> p c n", p=128))

    # ---------- persistent activation tiles ----------
    q_sb = apool.tile([128, 2, NS], BF)
    k_sb = apool.tile([128, 2, N], BF)
    # v^T per head + ones col, fp8, padded to stride 80 for DoubleRow
    vaT_sb = apool.tile([128, H, 16, 2, 80], F8)
    exp_sb = apool.tile([128, 2, 32, 512], F8)   # [., head-of-pair, m-chunk, n]
    attn_sb = apool.tile([128, 2, NS], BF)
    msg_sb = apool.tile([128, 2, NS], BF)
    h1_sb = apool.tile([128, 4, NS], FP)
    h1n_sb = apool.tile([128, 4, NS], BF)
    out_sb = apool.tile([128, 2, NS], FP)
    stats_sb = apool.tile([128, 8], FP)

    nc.vector.memset(vaT_sb[:, :, :, :, DH:DH + 1], 1.0)

    # ---------- phase 1: projections ----------
    with tc.tile_pool(name="srcp", bufs=1) as srcpool, \
         tc.tile_pool(name="pj", bufs=4, space="PSUM") as pj, \
         tc.tile_pool(name="vt", bufs=3, space="PSUM") as vtp:
        src_sb = srcpool.tile([128, 2, N], FP)
        nc.sync.dma_start(out=src_sb[:], in_=src.rearrange("(c p) m -> p c m", p=128))

        # q = WqT.T @ xs + bq   [256, NS]
        for oc in range(2):
            for ns in range(NS // 512):
                q_ps = pj.tile([128, 512], FP, tag="pj")
                for ic in range(2):
                    nc.tensor.matmul(
                        q_ps[:],
                        wq_sb[:, ic, oc * 128:(oc + 1) * 128],
                        xs_sb[:, ic, ns * 512:(ns + 1) * 512],
                        start=(ic == 0), stop=(ic == 1),
                    )
                nc.vector.tensor_scalar_add(
                    q_sb[:, oc, ns * 512:(ns + 1) * 512], q_ps[:], bq_sb[:, oc:oc + 1])

        # k = WkT.T @ src + bk   [256, N]
        for oc in range(2):
            for ns in range(N // 512):
                k_ps = pj.tile([128, 512], FP, tag="pj")
                for ic in range(2):
                    nc.tensor.matmul(
                        k_ps[:],
                        wk_sb[:, ic, oc * 128:(oc + 1) * 128],
                        src_sb[:, ic, ns * 512:(ns + 1) * 512],
                        start=(ic == 0), stop=(ic == 1),
                    )
                nc.vector.tensor_scalar_add(
                    k_sb[:, oc, ns * 512:(ns + 1) * 512], k_ps[:], bk_sb[:, oc:oc + 1])

        # v^T (+bias) directly transposed: out[m, c] = sum_i src[i, m] WvT[i, c]
        for mc in range(N // 128):
            vt_ps = vtp.tile([128, D], FP, tag="vt")
            for ic in range(2):
                nc.tensor.matmul(
                    vt_ps[:],
                    src_sb[:, ic, mc * 128:(mc + 1) * 128],
                    wv_sb[:, ic, :],
                    start=(ic == 0), stop=(ic == 1),
                )
            for h in range(H):
                nc.vector.tensor_add(
                    vaT_sb[:, h, mc // 2, mc % 2, 0:DH],
                    vt_ps[:, h * DH:(h + 1) * DH],
                    bvb_sb[:, h * DH:(h + 1) * DH],
                )

    if _STAGE == "proj":
        nc.vector.tensor_copy(out_sb[:], q_sb[:])
        nc.sync.dma_start(out=out.rearrange("(c p) n -> p c n", p=128), in_=out_sb[:])
        return

    # ---------- phase 2: attention ----------
    with tc.tile_pool(name="scA", bufs=1, space="PSUM") as scA, \
         tc.tile_pool(name="scB", bufs=1, space="PSUM") as scB, \
         tc.tile_pool(name="at", bufs=1, space="PSUM") as atp, \
         tc.tile_pool(name="nrm", bufs=4) as nrm:
        for hp in range(2):
            kc = hp
            for nch in range(NS // 512):
                n0 = nch * 512
                # scores_T[m, n] = k_h[:, m].T @ q_h[:, n] ; exp on ScalarE.
                # The pair's heads sit at base partitions 0/64, so adjacent
                # matmuls target disjoint PE row groups and run concurrently.
                for (j0, glen) in _JGROUPS:
                    pool = scA if glen == 2 else scB
                    sc_ps = pool.tile([128, 2, glen, 512], FP, tag=pool.name)
                    for j4 in range(glen):
                        j = j0 + j4
                        for hh in range(2):
                            bp = 64 * hh
                            nc.tensor.matmul(
                                sc_ps[:, hh, j4, :],
                                k_sb[bp:bp + DH, kc, j * 128:(j + 1) * 128],
                                q_sb[bp:bp + DH, kc, n0:n0 + 512],
                                start=True, stop=True,
                            )
                    nc.scalar.activation(
                        out=exp_sb[:, :, j0:j0 + glen, :], in_=sc_ps[:],
                        func=AF.Exp, scale=0.125)
                # attn (+Z) accumulate: out[0:64]=sum_m vT*exp, out[64]=sum_m exp
                # fp8 DoubleRow: two 128-row m-chunks per matmul pass
                for hh in range(2):
                    h, bp = 2 * hp + hh, 64 * hh
                    at_ps = atp.tile([128, 512], FP, tag=f"at{hh}")
                    for p in range(16):
                        nc.tensor.matmul(
                            at_ps[:DH + 1, :],
                            vaT_sb[:, h, p, :, 0:DH + 1],
                            exp_sb[:, hh, 2 * p:2 * p + 2, :],
                            start=(p == 0), stop=(p == 15),
                            perf_mode=mybir.MatmulPerfMode.DoubleRow,
                        )
                    rz = nrm.tile([1, 512], FP, tag="rz")
                    nc.vector.reciprocal(rz[:], at_ps[DH:DH + 1, :])
                    rzb = nrm.tile([DH, 512], FP, tag="rzb")
                    nc.gpsimd.partition_broadcast(rzb[:], rz[:])
                    nc.vector.tensor_mul(
                        attn_sb[bp:bp + DH, kc, n0:n0 + 512], at_ps[0:DH, :], rzb[:])

    if _STAGE == "attn":
        nc.vector.tensor_copy(out_sb[:], attn_sb[:])
        nc.sync.dma_start(out=out.rearrange("(c p) n -> p c n", p=128), in_=out_sb[:])
        return

    # ---------- phase 3: message, MLP, instance norm, output ----------
    with tc.tile_pool(name="mm", bufs=6, space="PSUM") as mm, \
         tc.tile_pool(name="dram", bufs=1, space="DRAM") as dram, \
         tc.tile_pool(name="nstat", bufs=1) as nstat:
        # message = WmT.T @ attn + bm
        for oc in range(2):
            for ns in range(NS // 512):
                m_ps = mm.tile([128, 512], FP, tag="mm")
                for ic in range(2):
                    nc.tensor.matmul(
                        m_ps[:],
                        wm_sb[:, ic, oc * 128:(oc + 1) * 128],
                        attn_sb[:, ic, ns * 512:(ns + 1) * 512],
                        start=(ic == 0), stop=(ic == 1),
                    )
                nc.vector.tensor_scalar_add(
                    msg_sb[:, oc, ns * 512:(ns + 1) * 512], m_ps[:], bm_sb[:, oc:oc + 1])

        if _STAGE == "msg":
            nc.vector.tensor_copy(out_sb[:], msg_sb[:])
            nc.sync.dma_start(out=out.rearrange("(c p) n -> p c n", p=128), in_=out_sb[:])
            return

        # h1 = W1T.T @ [xs; msg] + b1   [512, NS]
        for oc in range(4):
            for ns in range(NS // 512):
                h_ps = mm.tile([128, 512], FP, tag="mm")
                for ic in range(2):
                    nc.tensor.matmul(
                        h_ps[:],
                        w1x_sb[:, ic, oc * 128:(oc + 1) * 128],
                        xs_sb[:, ic, ns * 512:(ns + 1) * 512],
                        start=(ic == 0), stop=False,
                    )
                for ic in range(2):
                    nc.tensor.matmul(
                        h_ps[:],
                        w1m_sb[:, ic, oc * 128:(oc + 1) * 128],
                        msg_sb[:, ic, ns * 512:(ns + 1) * 512],
                        start=False, stop=(ic == 1),
                    )
                nc.vector.tensor_scalar_add(
                    h1_sb[:, oc, ns * 512:(ns + 1) * 512], h_ps[:], b1_sb[:, oc:oc + 1])

        if _STAGE == "h1":
            nc.vector.tensor_copy(out_sb[:, 0, :], h1_sb[:, 0, :])
            nc.vector.tensor_copy(out_sb[:, 1, :], h1_sb[:, 1, :])
            nc.sync.dma_start(out=out.rearrange("(c p) n -> p c n", p=128), in_=out_sb[:])
            return

        # per-core partial stats (sum, sumsq) over the local NS columns,
        # via bn_stats/bn_aggr (mean, biased var) -> scaled to (sum, sumsq)
        for t in range(4):
            bst = nstat.tile([128, 2, 6], FP, tag="bst")
            for g in range(2):
                nc.vector.bn_stats(out=bst[:, g, :], in_=h1_sb[:, t, g * 512:(g + 1) * 512])
            mv = nstat.tile([128, 2], FP, tag="mv")
            nc.vector.bn_aggr(out=mv[:], in_=bst[:])
            nc.vector.tensor_scalar_mul(stats_sb[:, t:t + 1], mv[:, 0:1], float(NS))
            msq = nstat.tile([128, 1], FP, tag="msq")
            nc.vector.tensor_mul(msq[:], mv[:, 0:1], mv[:, 0:1])
            msq2 = nstat.tile([128, 1], FP, tag="msq2")
            nc.vector.tensor_add(msq2[:], mv[:, 1:2], msq[:])
            nc.vector.tensor_scalar_mul(stats_sb[:, 4 + t:5 + t], msq2[:], float(NS))

        if _STAGE == "stats":
            nc.vector.tensor_copy(out_sb[:, 0, :], h1_sb[:, 0, :])
            nc.vector.tensor_copy(out_sb[:, 1, 0:8], stats_sb[:])
            nc.sync.dma_start(out=out.rearrange("(c p) n -> p c n", p=128), in_=out_sb[:])
            return

        # cross-core reduce within each batch group of 4 cores
        sred = nstat.tile([128, 8], FP)
        if _STAGE == "nocc":
            nc.vector.tensor_scalar_mul(sred[:], stats_sb[:], 4.0)
        else:
            cc_in = dram.tile([128, 8], FP)
            cc_out = dram.tile([128, 8], FP)
            nc.sync.dma_start(out=cc_in[:], in_=stats_sb[:])
            nc.gpsimd.collective_compute(
                "AllReduce", OP.add,
                replica_groups=[[0, 1, 2, 3], [4, 5, 6, 7]],
                ins=[cc_in[:].opt()], outs=[cc_out[:].opt()],
            )
            nc.sync.dma_start(out=sred[:], in_=cc_out[:])

        mu4 = nstat.tile([128, 4], FP)
        nc.vector.tensor_scalar_mul(mu4[:], sred[:, 0:4], 1.0 / N)
        e24 = nstat.tile([128, 4], FP)
        nc.vector.tensor_scalar_mul(e24[:], sred[:, 4:8], 1.0 / N)
        var4 = nstat.tile([128, 4], FP)
        nc.vector.tensor_mul(var4[:], mu4[:], mu4[:])
        nc.vector.tensor_tensor(out=var4[:], in0=e24[:], in1=var4[:], op=OP.subtract)
        eps1 = nstat.tile([128, 1], FP)
        nc.vector.memset(eps1[:], EPS)
        std4 = nstat.tile([128, 4], FP)
        nc.scalar.activation(out=std4[:], in_=var4[:], func=AF.Sqrt, bias=eps1[:])
        rstd4 = nstat.tile([128, 4], FP)
        nc.vector.reciprocal(rstd4[:], std4[:])
        nb4 = nstat.tile([128, 4], FP)
        nc.vector.tensor_mul(nb4[:], mu4[:], rstd4[:])
        nc.vector.tensor_scalar_mul(nb4[:], nb4[:], -1.0)

        # h = relu((h1 - mu) * rstd) = relu(h1 * rstd - mu * rstd)
        for t in range(4):
            nc.scalar.activation(
                out=h1n_sb[:, t, :], in_=h1_sb[:, t, :], func=AF.Relu,
                bias=nb4[:, t:t + 1], scale=rstd4[:, t:t + 1])

        # out = W2T.T @ h + b2
        for oc in range(2):
            for ns in range(NS // 512):
                o_ps = mm.tile([128, 512], FP, tag="mm")
                for kc2 in range(4):
                    nc.tensor.matmul(
                        o_ps[:],
                        w2_sb[:, kc2, oc * 128:(oc + 1) * 128],
                        h1n_sb[:, kc2, ns * 512:(ns + 1) * 512],
                        start=(kc2 == 0), stop=(kc2 == 3),
                    )
                nc.vector.tensor_scalar_add(
                    out_sb[:, oc, ns * 512:(ns + 1) * 512], o_ps[:], b2_sb[:, oc:oc + 1])

        nc.sync.dma_start(out=out.rearrange("(c p) n -> p c n", p=128), in_=out_sb[:])


_BUILT = {}


def _build():
    if "nc" in _BUILT:
        return _BUILT["nc"]
    nc = bacc.Bacc("TRN2", target_bir_lowering=False, debug=False,
                   enable_asserts=True, num_devices=NCORES)
    io = {}
    io["xs"] = nc.dram_tensor("xs", [D, NS], FP, kind="ExternalInput").ap()
    io["src"] = nc.dram_tensor("src", [D, N], FP, kind="ExternalInput").ap()
    io["wqT"] = nc.dram_tensor("wqT", [D, D], FP, kind="ExternalInput").ap()
    io["wkT"] = nc.dram_tensor("wkT", [D, D], FP, kind="ExternalInput").ap()
    io["wvT"] = nc.dram_tensor("wvT", [D, D], FP, kind="ExternalInput").ap()
    io["wmT"] = nc.dram_tensor("wmT", [D, D], FP, kind="ExternalInput").ap()
    io["w1xT"] = nc.dram_tensor("w1xT", [D, 2 * D], FP, kind="ExternalInput").ap()
    io["w1mT"] = nc.dram_tensor("w1mT", [D, 2 * D], FP, kind="ExternalInput").ap()
    io["w2T"] = nc.dram_tensor("w2T", [2 * D, D], FP, kind="ExternalInput").ap()
    io["bq"] = nc.dram_tensor("bq", [128, 2], FP, kind="ExternalInput").ap()
    io["bk"] = nc.dram_tensor("bk", [128, 2], FP, kind="ExternalInput").ap()
    io["bv"] = nc.dram_tensor("bv", [1, D], FP, kind="ExternalInput").ap()
    io["bm"] = nc.dram_tensor("bm", [128, 2], FP, kind="ExternalInput").ap()
    io["b1"] = nc.dram_tensor("b1", [128, 4], FP, kind="ExternalInput").ap()
    io["b2"] = nc.dram_tensor("b2", [128, 2], FP, kind="ExternalInput").ap()
    io["out"] = nc.dram_tensor("out", [D, NS], FP, kind="ExternalOutput").ap()

    import contextlib
    with tile.TileContext(nc) as tc:
        with contextlib.ExitStack() as es:
            _emit(nc, tc, io, es)
    nc.compile()
    _BUILT["nc"] = nc
    return nc


def _prep_inputs(x, source, Wq, bq, Wk, bk, Wv, bv, Wm, bm, W1, b1, W2, b2):
    perm = np.array([4 * d + h for h in range(H) for d in range(DH)])
    f32 = lambda a: np.ascontiguousarray(a, dtype=np.float32)

    shared = {
        "wqT": f32(Wq[perm, :].T),
        "wkT": f32(Wk[perm, :].T),
        "wvT": f32(Wv[perm, :].T),
        "wmT": f32(Wm[:, perm].T),
        "w1xT": f32(W1.T[0:D, :]),
        "w1mT": f32(W1.T[D:2 * D, :]),
        "w2T": f32(W2.T),
        "bq": f32(bq[perm].reshape(2, 128).T),
        "bk": f32(bk[perm].reshape(2, 128).T),
        "bv": f32(bv[perm].reshape(1, D)),
        "bm": f32(bm.reshape(2, 128).T),
        "b1": f32(b1.reshape(4, 128).T),
        "b2": f32(b2.reshape(2, 128).T),
    }
    in_maps = []
    for core in range(NCORES):
        b, s = core // 4, core % 4
        m = dict(shared)
        m["xs"] = f32(x[b][:, s * NS:(s + 1) * NS])
        m["src"] = f32(source[b])
        in_maps.append(m)
    return in_maps


def run(inputs, **spmd_kwargs):
    """Build (cached), run on cores 0-7, return (full_output, BassKernelResults)."""
    nc = _build()
    in_maps = _prep_inputs(**inputs)
    res = bass_utils.run_bass_kernel_spmd(
        nc, in_maps, core_ids=list(range(NCORES)), **spmd_kwargs)
    full = np.empty((B, D, N), dtype=np.float32)
    for core in range(NCORES):
        b, s = core // 4, core % 4
        full[b][:, s * NS:(s + 1) * NS] = res.results[core]["out"]
    return full, res


def kernel(**inputs):
    full, _ = run(inputs)
    return full



# revision 6
# speedup vs baseline: 1.2286x; 1.2286x over previous
# Trainium2 Bass kernel for nn_AttentionalPropagation (B=2, D=256, N=M=4096, H=4).
#
# Sharding: 8 cores; each batch (B=2) owns 4 cores; each core computes a
# 1024-column sequence shard of the output end-to-end (q/scores/softmax/attn/
# message/MLP). k,v are computed redundantly per core from the full `source`
# of its batch. The only cross-core communication is an AllReduce of the
# InstanceNorm partial (sum, sumsq) statistics within each 4-core batch group.
#
# Per-head layout trick: conv weights' output channels are permuted host-side
# so that head channels are contiguous on SBUF partitions (head h lives at
# partitions 64*(h%2) of channel-chunk h//2); this lets per-head matmuls run
# directly off partition-aligned slices (PE row/col tiling).
#
# Softmax: scores are built transposed ([m, n] with m on partitions), exp'd on
# ScalarE (scale=1/8 folded in, no max-subtraction needed: |s/8| < ~5), and the
# softmax denominator is obtained for free by augmenting v^T with a ones
# column in the attn matmul (out row 64 = sum_m exp). Normalization is a
# per-column reciprocal broadcast multiply.

import numpy as np

import concourse.bass as bass  # noqa: F401  (bass types used via tile/bacc)
import concourse.tile as tile
import concourse.mybir as mybir
from concourse import bacc
from concourse import bass_utils

B, D, N = 2, 256, 4096
H, DH = 4, 64
NS = N // 4           # sequence shard per core
NCORES = 8
EPS = 1e-5

FP = mybir.dt.float32
BF = mybir.dt.bfloat16
F8 = mybir.dt.float8e4
AX = mybir.AxisListType
OP = mybir.AluOpType
AF = mybir.ActivationFunctionType

# j-groups for the scores->exp pipeline. Each group's PSUM tile holds BOTH
# heads of the current pair (2 x glen x 512 fp32): glen=2 -> 4 banks,
# glen=1 -> 2 banks; the two tiles double-buffer within 6 free banks and the
# two per-head attn accumulators take the other 2.
_JGROUPS = []
_j = 0
while _j < 32:
    g = 2 if (len(_JGROUPS) % 2 == 0 and _j + 2 <= 32) else 1
    _JGROUPS.append((_j, g))
    _j += g

import os
_STAGE = os.environ.get("KSTAGE", "full")  # debug bisection: proj|attn|nocc|full


def _emit(nc, tc, io, es):
    xs, src = io["xs"], io["src"]
    out = io["out"]

    wpool = es.enter_context(tc.tile_pool(name="weights", bufs=1))
    apool = es.enter_context(tc.tile_pool(name="acts", bufs=1))

    # ---------- weight / bias loads ----------
    wq_sb = wpool.tile([128, 2, D], BF)
    nc.sync.dma_start(out=wq_sb[:], in_=io["wqT"].rearrange("(c p) o -> p c o", p=128))
    wk_sb = wpool.tile([128, 2, D], BF)
    nc.sync.dma_start(out=wk_sb[:], in_=io["wkT"].rearrange("(c p) o -> p c o", p=128))
    wv_sb = wpool.tile([128, 2, D], BF)
    nc.sync.dma_start(out=wv_sb[:], in_=io["wvT"].rearrange("(c p) o -> p c o", p=128))
    wm_sb = wpool.tile([128, 2, D], BF)
    nc.gpsimd.dma_start(out=wm_sb[:], in_=io["wmT"].rearrange("(c p) o -> p c o", p=128))
    w1x_sb = wpool.tile([128, 2, 2 * D], BF)
    nc.sync.dma_start(out=w1x_sb[:], in_=io["w1xT"].rearrange("(c p) o -> p c o", p=128))
    w1m_sb = wpool.tile([128, 2, 2 * D], BF)
    nc.gpsimd.dma_start(out=w1m_sb[:], in_=io["w1mT"].rearrange("(c p) o -> p c o", p=128))
    w2_sb = wpool.tile([128, 4, D], BF)
    nc.gpsimd.dma_start(out=w2_sb[:], in_=io["w2T"].rearrange("(c p) o -> p c o", p=128))

    bq_sb = wpool.tile([128, 2], FP)
    nc.sync.dma_start(out=bq_sb[:], in_=io["bq"][:])
    bk_sb = wpool.tile([128, 2], FP)
    nc.sync.dma_start(out=bk_sb[:], in_=io["bk"][:])
    bm_sb = wpool.tile([128, 2], FP)
    nc.sync.dma_start(out=bm_sb[:], in_=io["bm"][:])
    b1_sb = wpool.tile([128, 4], FP)
    nc.sync.dma_start(out=b1_sb[:], in_=io["b1"][:])
    b2_sb = wpool.tile([128, 2], FP)
    nc.sync.dma_start(out=b2_sb[:], in_=io["b2"][:])
    bv_sb = wpool.tile([1, D], FP)
    nc.sync.dma_start(out=bv_sb[:], in_=io["bv"][:])
    bvb_sb = wpool.tile([128, D], FP)
    nc.gpsimd.partition_broadcast(bvb_sb[:], bv_sb[:])

    xs_sb = apool.tile([128, 2, NS], BF)
    nc.sync.dma_start(out=xs_sb[:], in_=xs.rearrange("(c p) n -> p c n", p=128))

    # ---------- persistent activation tiles ----------
    q_sb = apool.tile([128, 2, NS], BF)
    k_sb = apool.tile([128, 2, N], BF)
    # v^T per head + ones col, fp8, padded to stride 80 for DoubleRow
    vaT_sb = apool.tile([128, H, 16, 2, 80], F8)
    exp_sb = apool.tile([128, 2, 32, 512], F8)   # [., head-of-pair, m-chunk, n]
    attn_sb = apool.tile([128, 2, NS], BF)
    msg_sb = apool.tile([128, 2, NS], BF)
    h1_sb = apool.tile([128, 4, NS], FP)
    h1n_sb = apool.tile([128, 4, NS], BF)
    out_sb = apool.tile([128, 2, NS], FP)
    stats_sb = apool.tile([128, 8], FP)

    nc.vector.memset(vaT_sb[:, :, :, :, DH:DH + 1], 1.0)

    # ---------- phase 1: projections ----------
    with tc.tile_pool(name="srcp", bufs=1) as srcpool, \
         tc.tile_pool(name="pj", bufs=4, space="PSUM") as pj, \
         tc.tile_pool(name="vt", bufs=3, space="PSUM") as vtp:
        src_sb = srcpool.tile([128, 2, N], BF)
        nc.sync.dma_start(out=src_sb[:], in_=src.rearrange("(c p) m -> p c m", p=128))

        # q = WqT.T @ xs + bq   [256, NS]
        for oc in range(2):
            for ns in range(NS // 512):
                q_ps = pj.tile([128, 512], FP, tag="pj")
                for ic in range(2):
                    nc.tensor.matmul(
                        q_ps[:],
                        wq_sb[:, ic, oc * 128:(oc + 1) * 128],
                        xs_sb[:, ic, ns * 512:(ns + 1) * 512],
                        start=(ic == 0), stop=(ic == 1),
                    )
                nc.vector.tensor_scalar_add(
                    q_sb[:, oc, ns * 512:(ns + 1) * 512], q_ps[:], bq_sb[:, oc:oc + 1])

        # k = WkT.T @ src + bk   [256, N]
        for oc in range(2):
            for ns in range(N // 512):
                k_ps = pj.tile([128, 512], FP, tag="pj")
                for ic in range(2):
                    nc.tensor.matmul(
                        k_ps[:],
                        wk_sb[:, ic, oc * 128:(oc + 1) * 128],
                        src_sb[:, ic, ns * 512:(ns + 1) * 512],
                        start=(ic == 0), stop=(ic == 1),
                    )
                nc.vector.tensor_scalar_add(
                    k_sb[:, oc, ns * 512:(ns + 1) * 512], k_ps[:], bk_sb[:, oc:oc + 1])

        # v^T (+bias) directly transposed: out[m, c] = sum_i src[i, m] WvT[i, c]
        for mc in range(N // 128):
            vt_ps = vtp.tile([128, D], FP, tag="vt")
            for ic in range(2):
                nc.tensor.matmul(
                    vt_ps[:],
                    src_sb[:, ic, mc * 128:(mc + 1) * 128],
                    wv_sb[:, ic, :],
                    start=(ic == 0), stop=(ic == 1),
                )
            for h in range(H):
                nc.vector.tensor_add(
                    vaT_sb[:, h, mc // 2, mc % 2, 0:DH],
                    vt_ps[:, h * DH:(h + 1) * DH],
                    bvb_sb[:, h * DH:(h + 1) * DH],
                )

    if _STAGE == "proj":
        nc.vector.tensor_copy(out_sb[:], q_sb[:])
        nc.sync.dma_start(out=out.rearrange("(c p) n -> p c n", p=128), in_=out_sb[:])
        return

    # ---------- phase 2: attention ----------
    with tc.tile_pool(name="scA", bufs=1, space="PSUM") as scA, \
         tc.tile_pool(name="scB", bufs=1, space="PSUM") as scB, \
         tc.tile_pool(name="at", bufs=1, space="PSUM") as atp, \
         tc.tile_pool(name="nrm", bufs=4) as nrm:
        for hp in range(2):
            kc = hp
            for nch in range(NS // 512):
                n0 = nch * 512
                # scores_T[m, n] = k_h[:, m].T @ q_h[:, n] ; exp on ScalarE.
                # The pair's heads sit at base partitions 0/64, so adjacent
                # matmuls target disjoint PE row groups and run concurrently.
                for (j0, glen) in _JGROUPS:
                    pool = scA if glen == 2 else scB
                    sc_ps = pool.tile([128, 2, glen, 512], FP, tag=pool.name)
                    for j4 in range(glen):
                        j = j0 + j4
                        for hh in range(2):
                            bp = 64 * hh
                            nc.tensor.matmul(
                                sc_ps[:, hh, j4, :],
                                k_sb[bp:bp + DH, kc, j * 128:(j + 1) * 128],
                                q_sb[bp:bp + DH, kc, n0:n0 + 512],
                                start=True, stop=True,
                            )
                    nc.scalar.activation(
                        out=exp_sb[:, :, j0:j0 + glen, :], in_=sc_ps[:],
                        func=AF.Exp, scale=0.125)
                # attn (+Z) accumulate: out[0:64]=sum_m vT*exp, out[64]=sum_m exp
                # fp8 DoubleRow: two 128-row m-chunks per matmul pass
                for hh in range(2):
                    h, bp = 2 * hp + hh, 64 * hh
                    at_ps = atp.tile([128, 512], FP, tag=f"at{hh}")
                    for p in range(16):
                        nc.tensor.matmul(
                            at_ps[:DH + 1, :],
                            vaT_sb[:, h, p, :, 0:DH + 1],
                            exp_sb[:, hh, 2 * p:2 * p + 2, :],
                            start=(p == 0), stop=(p == 15),
                            perf_mode=mybir.MatmulPerfMode.DoubleRow,
                        )
                    rz = nrm.tile([1, 512], FP, tag="rz")
                    nc.vector.reciprocal(rz[:], at_ps[DH:DH + 1, :])
                    rzb = nrm.tile([DH, 512], FP, tag="rzb")
                    nc.gpsimd.partition_broadcast(rzb[:], rz[:])
                    nc.vector.tensor_mul(
                        attn_sb[bp:bp + DH, kc, n0:n0 + 512], at_ps[0:DH, :], rzb[:])

    if _STAGE == "attn":
        nc.vector.tensor_copy(out_sb[:], attn_sb[:])
        nc.sync.dma_start(out=out.rearrange("(c p) n -> p c n", p=128), in_=out_sb[:])
        return

    # ---------- phase 3: message, MLP, instance norm, output ----------
    with tc.tile_pool(name="mm", bufs=6, space="PSUM") as mm, \
         tc.tile_pool(name="dram", bufs=1, space="DRAM") as dram, \
         tc.tile_pool(name="nstat", bufs=1) as nstat:
        # message = WmT.T @ attn + bm
        for oc in range(2):
            for ns in range(NS // 512):
                m_ps = mm.tile([128, 512], FP, tag="mm")
                for ic in range(2):
                    nc.tensor.matmul(
                        m_ps[:],
                        wm_sb[:, ic, oc * 128:(oc + 1) * 128],
                        attn_sb[:, ic, ns * 512:(ns + 1) * 512],
                        start=(ic == 0), stop=(ic == 1),
                    )
                nc.vector.tensor_scalar_add(
                    msg_sb[:, oc, ns * 512:(ns + 1) * 512], m_ps[:], bm_sb[:, oc:oc + 1])

        if _STAGE == "msg":
            nc.vector.tensor_copy(out_sb[:], msg_sb[:])
            nc.sync.dma_start(out=out.rearrange("(c p) n -> p c n", p=128), in_=out_sb[:])
            return

        # h1 = W1T.T @ [xs; msg] + b1   [512, NS]
        for oc in range(4):
            for ns in range(NS // 512):
                h_ps = mm.tile([128, 512], FP, tag="mm")
                for ic in range(2):
                    nc.tensor.matmul(
                        h_ps[:],
                        w1x_sb[:, ic, oc * 128:(oc + 1) * 128],
                        xs_sb[:, ic, ns * 512:(ns + 1) * 512],
                        start=(ic == 0), stop=False,
                    )
                for ic in range(2):
                    nc.tensor.matmul(
                        h_ps[:],
                        w1m_sb[:, ic, oc * 128:(oc + 1) * 128],
                        msg_sb[:, ic, ns * 512:(ns + 1) * 512],
                        start=False, stop=(ic == 1),
                    )
                nc.vector.tensor_scalar_add(
                    h1_sb[:, oc, ns * 512:(ns + 1) * 512], h_ps[:], b1_sb[:, oc:oc + 1])

        if _STAGE == "h1":
            nc.vector.tensor_copy(out_sb[:, 0, :], h1_sb[:, 0, :])
            nc.vector.tensor_copy(out_sb[:, 1, :], h1_sb[:, 1, :])
            nc.sync.dma_start(out=out.rearrange("(c p) n -> p c n", p=128), in_=out_sb[:])
            return

        # per-core partial stats (sum, sumsq) over the local NS columns,
        # via bn_stats/bn_aggr (mean, biased var) -> scaled to (sum, sumsq)
        for t in range(4):
            bst = nstat.tile([128, 2, 6], FP, tag="bst")
            for g in range(2):
                nc.vector.bn_stats(out=bst[:, g, :], in_=h1_sb[:, t, g * 512:(g + 1) * 512])
            mv = nstat.tile([128, 2], FP, tag="mv")
            nc.vector.bn_aggr(out=mv[:], in_=bst[:])
            nc.vector.tensor_scalar_mul(stats_sb[:, t:t + 1], mv[:, 0:1], float(NS))
            msq = nstat.tile([128, 1], FP, tag="msq")
            nc.vector.tensor_mul(msq[:], mv[:, 0:1], mv[:, 0:1])
            msq2 = nstat.tile([128, 1], FP, tag="msq2")
            nc.vector.tensor_add(msq2[:], mv[:, 1:2], msq[:])
            nc.vector.tensor_scalar_mul(stats_sb[:, 4 + t:5 + t], msq2[:], float(NS))

        if _STAGE == "stats":
            nc.vector.tensor_copy(out_sb[:, 0, :], h1_sb[:, 0, :])
            nc.vector.tensor_copy(out_sb[:, 1, 0:8], stats_sb[:])
            nc.sync.dma_start(out=out.rearrange("(c p) n -> p c n", p=128), in_=out_sb[:])
            return

        # cross-core reduce within each batch group of 4 cores
        sred = nstat.tile([128, 8], FP)
        if _STAGE == "nocc":
            nc.vector.tensor_scalar_mul(sred[:], stats_sb[:], 4.0)
        else:
            cc_in = dram.tile([128, 8], FP)
            cc_out = dram.tile([128, 8], FP)
            nc.sync.dma_start(out=cc_in[:], in_=stats_sb[:])
            nc.gpsimd.collective_compute(
                "AllReduce", OP.add,
                replica_groups=[[0, 1, 2, 3], [4, 5, 6, 7]],
                ins=[cc_in[:].opt()], outs=[cc_out[:].opt()],
            )
            nc.sync.dma_start(out=sred[:], in_=cc_out[:])

        mu4 = nstat.tile([128, 4], FP)
        nc.vector.tensor_scalar_mul(mu4[:], sred[:, 0:4], 1.0 / N)
        e24 = nstat.tile([128, 4], FP)
        nc.vector.tensor_scalar_mul(e24[:], sred[:, 4:8], 1.0 / N)
        var4 = nstat.tile([128, 4], FP)
        nc.vector.tensor_mul(var4[:], mu4[:], mu4[:])
        nc.vector.tensor_tensor(out=var4[:], in0=e24[:], in1=var4[:], op=OP.subtract)
        eps1 = nstat.tile([128, 1], FP)
        nc.vector.memset(eps1[:], EPS)
        std4 = nstat.tile([128, 4], FP)
        nc.scalar.activation(out=std4[:], in_=var4[:], func=AF.Sqrt, bias=eps1[:])
        rstd4 = nstat.tile([128, 4], FP)
        nc.vector.reciprocal(rstd4[:], std4[:])
        nb4 = nstat.tile([128, 4], FP)
        nc.vector.tensor_mul(nb4[:], mu4[:], rstd4[:])
        nc.vector.tensor_scalar_mul(nb4[:], nb4[:], -1.0)

        # h = relu((h1 - mu) * rstd) = relu(h1 * rstd - mu * rstd)
        for t in range(4):
            nc.scalar.activation(
                out=h1n_sb[:, t, :], in_=h1_sb[:, t, :], func=AF.Relu,
                bias=nb4[:, t:t + 1], scale=rstd4[:, t:t + 1])

        # out = W2T.T @ h + b2
        for oc in range(2):
            for ns in range(NS // 512):
                o_ps = mm.tile([128, 512], FP, tag="mm")
                for kc2 in range(4):
                    nc.tensor.matmul(
                        o_ps[:],
                        w2_sb[:, kc2, oc * 128:(oc + 1) * 128],
                        h1n_sb[:, kc2, ns * 512:(ns + 1) * 512],
                        start=(kc2 == 0), stop=(kc2 == 3),
                    )
                nc.vector.tensor_scalar_add(
                    out_sb[:, oc, ns * 512:(ns + 1) * 512], o_ps[:], b2_sb[:, oc:oc + 1])

        nc.sync.dma_start(out=out.rearrange("(c p) n -> p c n", p=128), in_=out_sb[:])


_BUILT = {}


def _build():
    if "nc" in _BUILT:
        return _BUILT["nc"]
    nc = bacc.Bacc("TRN2", target_bir_lowering=False, debug=False,
                   enable_asserts=True, num_devices=NCORES)
    io = {}
    io["xs"] = nc.dram_tensor("xs", [D, NS], BF, kind="ExternalInput").ap()
    io["src"] = nc.dram_tensor("src", [D, N], BF, kind="ExternalInput").ap()
    io["wqT"] = nc.dram_tensor("wqT", [D, D], BF, kind="ExternalInput").ap()
    io["wkT"] = nc.dram_tensor("wkT", [D, D], BF, kind="ExternalInput").ap()
    io["wvT"] = nc.dram_tensor("wvT", [D, D], BF, kind="ExternalInput").ap()
    io["wmT"] = nc.dram_tensor("wmT", [D, D], BF, kind="ExternalInput").ap()
    io["w1xT"] = nc.dram_tensor("w1xT", [D, 2 * D], BF, kind="ExternalInput").ap()
    io["w1mT"] = nc.dram_tensor("w1mT", [D, 2 * D], BF, kind="ExternalInput").ap()
    io["w2T"] = nc.dram_tensor("w2T", [2 * D, D], BF, kind="ExternalInput").ap()
    io["bq"] = nc.dram_tensor("bq", [128, 2], FP, kind="ExternalInput").ap()
    io["bk"] = nc.dram_tensor("bk", [128, 2], FP, kind="ExternalInput").ap()
    io["bv"] = nc.dram_tensor("bv", [1, D], FP, kind="ExternalInput").ap()
    io["bm"] = nc.dram_tensor("bm", [128, 2], FP, kind="ExternalInput").ap()
    io["b1"] = nc.dram_tensor("b1", [128, 4], FP, kind="ExternalInput").ap()
    io["b2"] = nc.dram_tensor("b2", [128, 2], FP, kind="ExternalInput").ap()
    io["out"] = nc.dram_tensor("out", [D, NS], FP, kind="ExternalOutput").ap()

    import contextlib
    with tile.TileContext(nc) as tc:
        with contextlib.ExitStack() as es:
            _emit(nc, tc, io, es)
    nc.compile()
    _BUILT["nc"] = nc
    return nc


def _prep_inputs(x, source, Wq, bq, Wk, bk, Wv, bv, Wm, bm, W1, b1, W2, b2):
    import ml_dtypes
    perm = np.array([4 * d + h for h in range(H) for d in range(DH)])
    f32 = lambda a: np.ascontiguousarray(a, dtype=np.float32)
    bf16 = lambda a: np.ascontiguousarray(
        np.asarray(a, dtype=np.float32), dtype=ml_dtypes.bfloat16)

    shared = {
        "wqT": bf16(Wq[perm, :].T),
        "wkT": bf16(Wk[perm, :].T),
        "wvT": bf16(Wv[perm, :].T),
        "wmT": bf16(Wm[:, perm].T),
        "w1xT": bf16(W1.T[0:D, :]),
        "w1mT": bf16(W1.T[D:2 * D, :]),
        "w2T": bf16(W2.T),
        "bq": f32(bq[perm].reshape(2, 128).T),
        "bk": f32(bk[perm].reshape(2, 128).T),
        "bv": f32(bv[perm].reshape(1, D)),
        "bm": f32(bm.reshape(2, 128).T),
        "b1": f32(b1.reshape(4, 128).T),
        "b2": f32(b2.reshape(2, 128).T),
    }
    in_maps = []
    for core in range(NCORES):
        b, s = core // 4, core % 4
        m = dict(shared)
        m["xs"] = bf16(x[b][:, s * NS:(s + 1) * NS])
        m["src"] = bf16(source[b])
        in_maps.append(m)
    return in_maps


def run(inputs, **spmd_kwargs):
    """Build (cached), run on cores 0-7, return (full_output, BassKernelResults)."""
    nc = _build()
    in_maps = _prep_inputs(**inputs)
    res = bass_utils.run_bass_kernel_spmd(
        nc, in_maps, core_ids=list(range(NCORES)), **spmd_kwargs)
    full = np.empty((B, D, N), dtype=np.float32)
    for core in range(NCORES):
        b, s = core // 4, core % 4
        full[b][:, s * NS:(s + 1) * NS] = res.results[core]["out"]
    return full, res


def kernel(**inputs):
    full, _ = run(inputs)
    return full



# revision 21
# speedup vs baseline: 1.2705x; 1.0341x over previous
# Trainium2 Bass kernel for nn_AttentionalPropagation (B=2, D=256, N=M=4096, H=4).
#
# Sharding: 8 cores; each batch (B=2) owns 4 cores; each core computes a
# 1024-column sequence shard of the output end-to-end. k,v are computed
# redundantly per core from the full `source` of its batch. Cross-core
# communication: one AllGather of InstanceNorm partial (sum, sumsq) stats
# within each 4-core batch group (+ local sum), cheaper in the cost model
# than AllReduce.
#
# Engine plan (per core, cost-model driven):
#  - PE: all projections as fp8 DoubleRow (host-side fp8 conversion of
#    x/source/Wq/Wk/Wv), fp8 DoubleRow scores, mixed fp8-DoubleRow/bf16
#    attention, bf16 msg/h1/out. Biases enter via DVE evictions (q, v),
#    hi/lo bf16 ones-row matmul passes (b1', b2), or cancel entirely
#    (bk shifts each softmax column by a constant over m -> dropped;
#    bm is folded into b1' host-side).
#  - Softmax: scores/8 - 1 (offset keeps fp8 exp in range; cancels in the
#    normalization). exp is split between ACT (fp8 output, feeds DoubleRow
#    attn passes) and DVE (Schraudolph int16 bit-trick -> bf16 probs, feeds
#    plain bf16 attn passes). Denominator via the ones-column in v^T.
#  - Evictions that need no arithmetic go over DMA queues (msg, h1, out).

import os

import numpy as np

import concourse.bass as bass  # noqa: F401
import concourse.tile as tile
import concourse.mybir as mybir
from concourse import bacc
from concourse import bass_utils

B, D, N = 2, 256, 4096
H, DH = 4, 64
NS = N // 4           # sequence shard per core
NCORES = 8
EPS = 1e-5

FP = mybir.dt.float32
BF = mybir.dt.bfloat16
F8 = mybir.dt.float8e4
I16 = mybir.dt.int16
OP = mybir.AluOpType
AF = mybir.ActivationFunctionType
DR = mybir.MatmulPerfMode.DoubleRow

JA = 20               # exp chunks per group on ACT (fp8, DoubleRow attn)
JD = 32 - JA          # exp chunks per group on DVE (Schraudolph bf16)
C0 = 1.0              # exp offset: probs = exp(s/8 - C0); cancels in softmax
LOG2E = 1.4426950408889634
SCH_A = 0.125 * 128 * LOG2E                       # i16 = s*SCH_A + SCH_B
SCH_B = 127.0 * 128 - 128 * C0 * LOG2E - 0.5      # -0.5 centers truncation

_STAGE = os.environ.get("KSTAGE", "full")  # debug bisection: qk|attn|h1|full
_ALLGATHER = os.environ.get("KAG", "1") == "1"  # stats exchange: AllGather vs AllReduce


def _emit(nc, tc, io, es):
    out = io["out"]

    wpool = es.enter_context(tc.tile_pool(name="weights", bufs=1))
    apool = es.enter_context(tc.tile_pool(name="acts", bufs=1))

    # ---------- weight / bias / input loads ----------
    wq_sb = wpool.tile([128, 2, D], F8)
    nc.sync.dma_start(out=wq_sb[:], in_=io["wq8"].rearrange("(c p) o -> p c o", p=128))
    wk_sb = wpool.tile([128, 2, D], F8)
    nc.sync.dma_start(out=wk_sb[:], in_=io["wk8"].rearrange("(c p) o -> p c o", p=128))
    wv_sb = wpool.tile([128, 2, D], F8)
    nc.sync.dma_start(out=wv_sb[:], in_=io["wv8"].rearrange("(c p) o -> p c o", p=128))
    wm_sb = wpool.tile([128, 2, D], BF)
    nc.gpsimd.dma_start(out=wm_sb[:], in_=io["wmT"].rearrange("(c p) o -> p c o", p=128))
    w1x_sb = wpool.tile([128, 2, 2 * D], BF)
    nc.sync.dma_start(out=w1x_sb[:], in_=io["w1xT"].rearrange("(c p) o -> p c o", p=128))
    w1m_sb = wpool.tile([128, 2, 2 * D], BF)
    nc.gpsimd.dma_start(out=w1m_sb[:], in_=io["w1mT"].rearrange("(c p) o -> p c o", p=128))
    w2_sb = wpool.tile([128, 4, D], BF)
    nc.gpsimd.dma_start(out=w2_sb[:], in_=io["w2T"].rearrange("(c p) o -> p c o", p=128))

    bq_sb = wpool.tile([128, 2], FP)
    nc.sync.dma_start(out=bq_sb[:], in_=io["bq"][:])
    bvb_sb = wpool.tile([128, 4, 2, DH], FP)
    nc.sync.dma_start(
        out=bvb_sb[:], in_=io["bvb"].rearrange("p (h r d) -> p h r d", h=4, r=2))
    b1hl_sb = wpool.tile([2, 2 * D], BF)
    nc.sync.dma_start(out=b1hl_sb[:], in_=io["b1hl"][:])
    b2hl_sb = wpool.tile([2, D], BF)
    nc.sync.dma_start(out=b2hl_sb[:], in_=io["b2hl"][:])

    xs8_sb = apool.tile([128, 2, NS], F8)
    nc.sync.dma_start(out=xs8_sb[:], in_=io["xs8"].rearrange("(c p) n -> p c n", p=128))
    xsb_sb = apool.tile([128, 2, NS], BF)
    nc.sync.dma_start(out=xsb_sb[:], in_=io["xsb"].rearrange("(c p) n -> p c n", p=128))
    src8_sb = apool.tile([128, 2, N], F8)
    nc.sync.dma_start(out=src8_sb[:], in_=io["src8"].rearrange("(c p) m -> p c m", p=128))

    ones2 = wpool.tile([2, 512], BF)
    nc.vector.memset(ones2[:], 1.0)
    negc0 = wpool.tile([128, 1], FP)
    nc.vector.memset(negc0[:], -C0)

    # ---------- persistent activation tiles ----------
    qf_sb = apool.tile([128, 2, NS], F8)      # part = 64*hh + d, [kc, n]
    kf_sb = apool.tile([128, 2, N], F8)
    # DoubleRow score layout: one 32-partition tile per (kc, hh), dims [p, r, n]
    q8_t = [[apool.tile([32, 2, NS], F8, name=f"q8_{kc}{hh}") for hh in range(2)]
            for kc in range(2)]
    k8_t = [[apool.tile([32, 2, N], F8, name=f"k8_{kc}{hh}") for hh in range(2)]
            for kc in range(2)]
    # v^T per head + ones col, fp8, stride 80 for DoubleRow
    vaT_sb = apool.tile([128, H, 16, 2, 80], F8)
    exp8_sb = apool.tile([128, 2, JA // 2, 2, 512], F8)   # [., hh, p, r, n]
    prob16_sb = apool.tile([128, 2, JD, 512], I16)        # [., hh, jd, n]
    attn_sb = apool.tile([128, 2, NS], BF)
    msg_sb = apool.tile([128, 2, NS], BF)
    h1_sb = apool.tile([128, 4, NS], FP)
    h1n_sb = apool.tile([128, 4, NS], BF)
    bst_sb = apool.tile([128, 4, 2, 6], FP)
    stats_sb = apool.tile([128, 8], FP)

    nc.vector.memset(vaT_sb[:, :, :, :, DH:DH + 1], 1.0)

    # ---------- phase 1: projections (all fp8 DoubleRow) ----------
    with tc.tile_pool(name="pj", bufs=2, space="PSUM") as pj, \
         tc.tile_pool(name="vt", bufs=2, space="PSUM") as vtp:
        # q: per kc one DoubleRow matmul (contract 256 = 128 part x 2 ic)
        for kc in range(2):
            q_ps = pj.tile([128, NS], FP, tag="pj")
            for nh in range(2):
                nc.tensor.matmul(
                    q_ps[:, nh * 512:(nh + 1) * 512],
                    wq_sb[:, :, kc * 128:(kc + 1) * 128],
                    xs8_sb[:, :, nh * 512:(nh + 1) * 512],
                    start=True, stop=True, perf_mode=DR)
            nc.vector.tensor_scalar(
                out=qf_sb[:, kc, :], in0=q_ps[:],
                scalar1=bq_sb[:, kc:kc + 1], scalar2=None, op0=OP.add)
        # k: per (kc, m-chunk); no bias (constant-over-m shift cancels in softmax)
        for kc in range(2):
            for mq in range(4):
                k_ps = pj.tile([128, NS], FP, tag="pj")
                for mh in range(2):
                    m0 = mq * NS + mh * 512
                    nc.tensor.matmul(
                        k_ps[:, mh * 512:(mh + 1) * 512],
                        wk_sb[:, :, kc * 128:(kc + 1) * 128],
                        src8_sb[:, :, m0:m0 + 512],
                        start=True, stop=True, perf_mode=DR)
                nc.vector.tensor_copy(kf_sb[:, kc, mq * NS:(mq + 1) * NS], k_ps[:])
        # v^T: per m-chunk of 128; out[m, (h d)]
        for g in range(8):
            vt_ps = vtp.tile([128, 2, 2, 4, DH], FP, tag="vt")
            for mm in range(4):
                mc = 4 * g + mm
                nc.tensor.matmul(
                    vt_ps[:, mm // 2, mm % 2, :, :],
                    src8_sb[:, :, mc * 128:(mc + 1) * 128], wv_sb[:],
                    start=True, stop=True, perf_mode=DR)
            for pb in range(2):
                nc.vector.tensor_tensor(
                    out=vaT_sb[:, :, 2 * g + pb, :, 0:DH],
                    in0=vt_ps[:, pb, :, :, :].transpose([0, 2, 1, 3]),
                    in1=bvb_sb[:], op=OP.add)

    # reshuffle q/k to DoubleRow score layout (partition regrouping via DMA)
    for kc in range(2):
        for hh in range(2):
            pi = 64 * hh
            for r in range(2):
                nc.gpsimd.dma_start(
                    out=q8_t[kc][hh][:, r, :], in_=qf_sb[pi + 32 * r:pi + 32 * r + 32, kc, :])
                nc.gpsimd.dma_start(
                    out=k8_t[kc][hh][:, r, :], in_=kf_sb[pi + 32 * r:pi + 32 * r + 32, kc, :])

    if _STAGE == "qk":
        o_dbg = apool.tile([128, 2, NS], FP)
        nc.vector.tensor_copy(o_dbg[:, 0, :], qf_sb[:, 0, :])
        nc.vector.tensor_copy(o_dbg[:, 1, :], qf_sb[:, 1, :])
        nc.sync.dma_start(out=out.rearrange("(c p) n -> p c n", p=128), in_=o_dbg[:])
        return

    # ---------- phase 2+3: attention + chunkwise MLP pipeline ----------
    with tc.tile_pool(name="sc", bufs=2, space="PSUM") as scp, \
         tc.tile_pool(name="at", bufs=1, space="PSUM") as atp, \
         tc.tile_pool(name="mm", bufs=2, space="PSUM") as mmp, \
         tc.tile_pool(name="nrm", bufs=2) as nrm:

        def emit_mlp(nch):
            n0 = nch * 512
            # message (bm folded into b1' host-side; 1/Z applied at attn evict)
            for oc in range(2):
                m_ps = mmp.tile([128, 512], FP, tag="mm")
                for ic in range(2):
                    nc.tensor.matmul(
                        m_ps[:], wm_sb[:, ic, oc * 128:(oc + 1) * 128],
                        attn_sb[:, ic, n0:n0 + 512],
                        start=(ic == 0), stop=(ic == 1))
                nc.vector.tensor_copy(msg_sb[:, oc, n0:n0 + 512], m_ps[:])
            # h1 = W1x@x + W1m@msg + b1' (ones-row hi/lo pass)
            for t in range(4):
                h_ps = mmp.tile([128, 512], FP, tag="mm")
                for ic in range(2):
                    nc.tensor.matmul(
                        h_ps[:], w1x_sb[:, ic, t * 128:(t + 1) * 128],
                        xsb_sb[:, ic, n0:n0 + 512], start=(ic == 0), stop=False)
                for ic in range(2):
                    nc.tensor.matmul(
                        h_ps[:], w1m_sb[:, ic, t * 128:(t + 1) * 128],
                        msg_sb[:, ic, n0:n0 + 512], start=False, stop=False)
                nc.tensor.matmul(
                    h_ps[:], b1hl_sb[:, t * 128:(t + 1) * 128], ones2[:],
                    start=False, stop=True)
                nc.vector.tensor_copy(h1_sb[:, t, n0:n0 + 512], h_ps[:])
                nc.vector.bn_stats(out=bst_sb[:, t, nch, :], in_=h1_sb[:, t, n0:n0 + 512])

        for nch in range(2):
            n0 = nch * 512
            for kc in range(2):
                at0 = atp.tile([128, 512], FP, tag="at0")
                at1 = atp.tile([128, 512], FP, tag="at1")
                ats = (at0, at1)
                for j in range(32):
                    sc_ps = scp.tile([128, 2, 512], FP, tag="sc")
                    for hh in range(2):
                        nc.tensor.matmul(
                            sc_ps[:, hh, :],
                            k8_t[kc][hh][:, :, j * 128:(j + 1) * 128],
                            q8_t[kc][hh][:, :, n0:n0 + 512],
                            start=True, stop=True, perf_mode=DR)
                    if j < JA:
                        nc.scalar.activation(
                            out=exp8_sb[:, :, j // 2, j % 2, :], in_=sc_ps[:],
                            func=AF.Exp, scale=0.125, bias=negc0[:])
                    else:
                        nc.vector.tensor_scalar(
                            out=prob16_sb[:, :, j - JA, :], in0=sc_ps[:],
                            scalar1=SCH_A, scalar2=SCH_B, op0=OP.mult, op1=OP.add)
                for hh in range(2):
                    h = 2 * kc + hh
                    at = ats[hh]
                    for p in range(JA // 2):
                        nc.tensor.matmul(
                            at[:DH + 1, :], vaT_sb[:, h, p, :, 0:DH + 1],
                            exp8_sb[:, hh, p, :, :],
                            start=(p == 0), stop=False, perf_mode=DR)
                    for jd in range(JD):
                        j = JA + jd
                        nc.tensor.matmul(
                            at[:DH + 1, :], vaT_sb[:, h, j // 2, j % 2, 0:DH + 1],
                            prob16_sb[:, hh, jd, :].bitcast(BF),
                            start=False, stop=(jd == JD - 1))
                for hh in range(2):
                    at = ats[hh]
                    rz = nrm.tile([1, 512], FP, tag="rz")
                    nc.vector.reciprocal(rz[:], at[DH:DH + 1, :])
                    rzb = nrm.tile([DH, 512], FP, tag="rzb")
                    nc.gpsimd.partition_broadcast(rzb[:], rz[:])
                    nc.vector.tensor_mul(
                        attn_sb[64 * hh:64 * hh + DH, kc, n0:n0 + 512],
                        at[0:DH, :], rzb[:])
            emit_mlp(nch)

        if _STAGE == "attn":
            o_dbg = apool.tile([128, 2, NS], FP)
            nc.vector.tensor_copy(o_dbg[:, 0, :], attn_sb[:, 0, :])
            nc.vector.tensor_copy(o_dbg[:, 1, :], attn_sb[:, 1, :])
            nc.sync.dma_start(out=out.rearrange("(c p) n -> p c n", p=128), in_=o_dbg[:])
            return
        if _STAGE == "h1":
            o_dbg = apool.tile([128, 2, NS], FP)
            nc.vector.tensor_copy(o_dbg[:, 0, :], h1_sb[:, 0, :])
            nc.vector.tensor_copy(o_dbg[:, 1, :], h1_sb[:, 1, :])
            nc.sync.dma_start(out=out.rearrange("(c p) n -> p c n", p=128), in_=o_dbg[:])
            return

        # ---------- instance-norm stats exchange + output ----------
        with tc.tile_pool(name="dram", bufs=1, space="DRAM") as dram, \
             tc.tile_pool(name="nstat", bufs=1) as nstat:
            # local (sum, sumsq) per channel from bn stats
            for t in range(4):
                mv = nstat.tile([128, 2], FP, tag="mv")
                nc.vector.bn_aggr(out=mv[:], in_=bst_sb[:, t, :, :])
                nc.vector.tensor_scalar_mul(stats_sb[:, t:t + 1], mv[:, 0:1], float(NS))
                msq = nstat.tile([128, 1], FP, tag="msq")
                nc.vector.tensor_mul(msq[:], mv[:, 0:1], mv[:, 0:1])
                msq2 = nstat.tile([128, 1], FP, tag="msq2")
                nc.vector.tensor_add(msq2[:], mv[:, 1:2], msq[:])
                nc.vector.tensor_scalar_mul(stats_sb[:, 4 + t:5 + t], msq2[:], float(NS))

            cc_in = dram.tile([128, 8], FP)
            if _ALLGATHER:
                cc_out = dram.tile([4, 128, 8], FP)
                nc.sync.dma_start(out=cc_in[:], in_=stats_sb[:])
                nc.gpsimd.collective_compute(
                    "AllGather", OP.bypass,
                    replica_groups=[[0, 1, 2, 3], [4, 5, 6, 7]],
                    ins=[cc_in[:].opt()], outs=[cc_out[:].opt()],
                )
                sred4 = nstat.tile([128, 4, 8], FP)
                nc.sync.dma_start(out=sred4[:], in_=cc_out.rearrange("k p s -> p k s"))
                s01 = nstat.tile([128, 8], FP)
                nc.vector.tensor_add(s01[:], sred4[:, 0, :], sred4[:, 1, :])
                s23 = nstat.tile([128, 8], FP)
                nc.vector.tensor_add(s23[:], sred4[:, 2, :], sred4[:, 3, :])
                sred = nstat.tile([128, 8], FP)
                nc.vector.tensor_add(sred[:], s01[:], s23[:])
            else:
                cc_out = dram.tile([128, 8], FP)
                nc.sync.dma_start(out=cc_in[:], in_=stats_sb[:])
                nc.gpsimd.collective_compute(
                    "AllReduce", OP.add,
                    replica_groups=[[0, 1, 2, 3], [4, 5, 6, 7]],
                    ins=[cc_in[:].opt()], outs=[cc_out[:].opt()],
                )
                sred = nstat.tile([128, 8], FP)
                nc.sync.dma_start(out=sred[:], in_=cc_out[:])

            mu4 = nstat.tile([128, 4], FP)
            nc.vector.tensor_scalar_mul(mu4[:], sred[:, 0:4], 1.0 / N)
            e24 = nstat.tile([128, 4], FP)
            nc.vector.tensor_scalar_mul(e24[:], sred[:, 4:8], 1.0 / N)
            var4 = nstat.tile([128, 4], FP)
            nc.vector.tensor_mul(var4[:], mu4[:], mu4[:])
            nc.vector.tensor_tensor(out=var4[:], in0=e24[:], in1=var4[:], op=OP.subtract)
            eps1 = nstat.tile([128, 1], FP)
            nc.vector.memset(eps1[:], EPS)
            std4 = nstat.tile([128, 4], FP)
            nc.scalar.activation(out=std4[:], in_=var4[:], func=AF.Sqrt, bias=eps1[:])
            rstd4 = nstat.tile([128, 4], FP)
            nc.vector.reciprocal(rstd4[:], std4[:])
            nb4 = nstat.tile([128, 4], FP)
            nc.vector.tensor_mul(nb4[:], mu4[:], rstd4[:])
            nc.vector.tensor_scalar_mul(nb4[:], nb4[:], -1.0)

            # h = relu(h1 * rstd - mu * rstd)
            for t in range(4):
                nc.scalar.activation(
                    out=h1n_sb[:, t, :], in_=h1_sb[:, t, :], func=AF.Relu,
                    bias=nb4[:, t:t + 1], scale=rstd4[:, t:t + 1])

            # out = W2T.T @ h + b2 (ones-row hi/lo pass)
            out_sb = apool.tile([128, 2, NS], FP)
            outp = out.rearrange("(c p) n -> p c n", p=128)
            for oc in range(2):
                for nch in range(2):
                    n0 = nch * 512
                    o_ps = mmp.tile([128, 512], FP, tag="mm")
                    for kc2 in range(4):
                        nc.tensor.matmul(
                            o_ps[:], w2_sb[:, kc2, oc * 128:(oc + 1) * 128],
                            h1n_sb[:, kc2, n0:n0 + 512],
                            start=(kc2 == 0), stop=False)
                    nc.tensor.matmul(
                        o_ps[:], b2hl_sb[:, oc * 128:(oc + 1) * 128], ones2[:],
                        start=False, stop=True)
                    nc.vector.tensor_copy(out_sb[:, oc, n0:n0 + 512], o_ps[:])
                    nc.sync.dma_start(out=outp[:, oc, n0:n0 + 512], in_=out_sb[:, oc, n0:n0 + 512])


_BUILT = {}


def _build():
    if "nc" in _BUILT:
        return _BUILT["nc"]
    nc = bacc.Bacc("TRN2", target_bir_lowering=False, debug=False,
                   enable_asserts=True, num_devices=NCORES)
    io = {}
    io["xs8"] = nc.dram_tensor("xs8", [D, NS], F8, kind="ExternalInput").ap()
    io["xsb"] = nc.dram_tensor("xsb", [D, NS], BF, kind="ExternalInput").ap()
    io["src8"] = nc.dram_tensor("src8", [D, N], F8, kind="ExternalInput").ap()
    io["wq8"] = nc.dram_tensor("wq8", [D, D], F8, kind="ExternalInput").ap()
    io["wk8"] = nc.dram_tensor("wk8", [D, D], F8, kind="ExternalInput").ap()
    io["wv8"] = nc.dram_tensor("wv8", [D, D], F8, kind="ExternalInput").ap()
    io["wmT"] = nc.dram_tensor("wmT", [D, D], BF, kind="ExternalInput").ap()
    io["w1xT"] = nc.dram_tensor("w1xT", [D, 2 * D], BF, kind="ExternalInput").ap()
    io["w1mT"] = nc.dram_tensor("w1mT", [D, 2 * D], BF, kind="ExternalInput").ap()
    io["w2T"] = nc.dram_tensor("w2T", [2 * D, D], BF, kind="ExternalInput").ap()
    io["bq"] = nc.dram_tensor("bq", [128, 2], FP, kind="ExternalInput").ap()
    io["bvb"] = nc.dram_tensor("bvb", [128, 8 * DH], FP, kind="ExternalInput").ap()
    io["b1hl"] = nc.dram_tensor("b1hl", [2, 2 * D], BF, kind="ExternalInput").ap()
    io["b2hl"] = nc.dram_tensor("b2hl", [2, D], BF, kind="ExternalInput").ap()
    io["out"] = nc.dram_tensor("out", [D, NS], FP, kind="ExternalOutput").ap()

    import contextlib
    with tile.TileContext(nc) as tc:
        with contextlib.ExitStack() as es:
            _emit(nc, tc, io, es)
    nc.compile()
    _BUILT["nc"] = nc
    return nc


def _prep_inputs(x, source, Wq, bq, Wk, bk, Wv, bv, Wm, bm, W1, b1, W2, b2):
    import ml_dtypes
    npF8 = mybir.dt.np(F8)
    npBF = ml_dtypes.bfloat16
    perm = np.array([4 * d + h for h in range(H) for d in range(DH)])
    f32 = lambda a: np.ascontiguousarray(a, dtype=np.float32)
    bf = lambda a: np.ascontiguousarray(np.asarray(a, np.float32), dtype=npBF)
    f8c = lambda a: np.ascontiguousarray(np.asarray(a, np.float32), dtype=npF8)

    b1p = np.asarray(b1, np.float64) + np.asarray(W1, np.float64)[:, D:] @ np.asarray(bm, np.float64)
    b1hi = np.asarray(b1p, np.float32).astype(npBF)
    b1lo = (np.asarray(b1p, np.float32) - b1hi.astype(np.float32)).astype(npBF)
    b2hi = np.asarray(b2, np.float32).astype(npBF)
    b2lo = (np.asarray(b2, np.float32) - b2hi.astype(np.float32)).astype(npBF)

    bvp = np.asarray(bv, np.float32)[perm]                    # (h, d) order
    bvb = np.tile(bvp.reshape(4, 1, DH), (1, 2, 1)).reshape(1, 8 * DH)
    bvb = np.ascontiguousarray(np.broadcast_to(bvb, (128, 8 * DH)), np.float32)

    shared = {
        "wq8": f8c(Wq[perm, :].T),
        "wk8": f8c(Wk[perm, :].T),
        "wv8": f8c(Wv[perm, :].T),
        "wmT": bf(Wm[:, perm].T),
        "w1xT": bf(W1.T[0:D, :]),
        "w1mT": bf(W1.T[D:2 * D, :]),
        "w2T": bf(W2.T),
        "bq": f32(bq[perm].reshape(2, 128).T),
        "bvb": bvb,
        "b1hl": np.ascontiguousarray(np.stack([b1hi, b1lo])),
        "b2hl": np.ascontiguousarray(np.stack([b2hi, b2lo])),
    }
    in_maps = []
    for core in range(NCORES):
        b, s = core // 4, core % 4
        m = dict(shared)
        xs = x[b][:, s * NS:(s + 1) * NS]
        m["xs8"] = f8c(xs)
        m["xsb"] = bf(xs)
        m["src8"] = f8c(source[b])
        in_maps.append(m)
    return in_maps


def run(inputs, **spmd_kwargs):
    """Build (cached), run on cores 0-7, return (full_output, BassKernelResults)."""
    nc = _build()
    in_maps = _prep_inputs(**inputs)
    res = bass_utils.run_bass_kernel_spmd(
        nc, in_maps, core_ids=list(range(NCORES)), **spmd_kwargs)
    full = np.empty((B, D, N), dtype=np.float32)
    for core in range(NCORES):
        b, s = core // 4, core % 4
        full[b][:, s * NS:(s + 1) * NS] = res.results[core]["out"]
    return full, res


def kernel(**inputs):
    full, _ = run(inputs)
    return full


# revision 26
# speedup vs baseline: 1.3356x; 1.0512x over previous
# Trainium2 Bass kernel for nn_AttentionalPropagation (B=2, D=256, N=M=4096, H=4).
#
# Sharding: 8 cores; each batch (B=2) owns 4 cores; each core computes a
# 1024-column sequence shard of the output end-to-end. k,v are computed
# redundantly per core from the full `source` of its batch. Cross-core
# communication: one AllGather of InstanceNorm partial (sum, sumsq) stats
# within each 4-core batch group (+ local sum), cheaper in the cost model
# than AllReduce.
#
# Engine plan (per core, cost-model driven):
#  - PE: all projections as fp8 DoubleRow (host-side fp8 conversion of
#    x/source/Wq/Wk/Wv), fp8 DoubleRow scores, mixed fp8-DoubleRow/bf16
#    attention, bf16 msg/h1/out. Biases enter via DVE evictions (q, v),
#    hi/lo bf16 ones-row matmul passes (b1', b2), or cancel entirely
#    (bk shifts each softmax column by a constant over m -> dropped;
#    bm is folded into b1' host-side).
#  - Softmax: scores/8 - 1 (offset keeps fp8 exp in range; cancels in the
#    normalization). exp is split between ACT (fp8 output, feeds DoubleRow
#    attn passes) and DVE (Schraudolph int16 bit-trick -> bf16 probs, feeds
#    plain bf16 attn passes). Denominator via the ones-column in v^T.
#  - Evictions that need no arithmetic go over DMA queues (msg, h1, out).

import os

import numpy as np

import concourse.bass as bass  # noqa: F401
import concourse.tile as tile
import concourse.mybir as mybir
from concourse import bacc
from concourse import bass_utils

B, D, N = 2, 256, 4096
H, DH = 4, 64
NS = N // 4           # sequence shard per core
NCORES = 8
EPS = 1e-5

FP = mybir.dt.float32
BF = mybir.dt.bfloat16
F8 = mybir.dt.float8e4
I16 = mybir.dt.int16
OP = mybir.AluOpType
AF = mybir.ActivationFunctionType
DR = mybir.MatmulPerfMode.DoubleRow

JA = 20               # exp chunks per group on ACT (fp8, DoubleRow attn)
JD = 32 - JA          # exp chunks per group on DVE (Schraudolph bf16)
# Interleave ACT/DVE ownership over the 16 m-chunk PAIRS of a group so both
# engines run concurrently (ACT pairs feed DoubleRow attn; DVE pairs bf16).
_NPAIR, _NDP = 16, JD // 2
_DVE_PAIRS = sorted({int(round((i + 0.5) * _NPAIR / _NDP - 0.5)) for i in range(_NDP)})
assert len(_DVE_PAIRS) == _NDP
C0 = 1.0              # exp offset: probs = exp(s/8 - C0); cancels in softmax
LOG2E = 1.4426950408889634
SCH_A = 0.125 * 128 * LOG2E                       # i16 = s*SCH_A + SCH_B
SCH_B = 127.0 * 128 - 128 * C0 * LOG2E - 0.5      # -0.5 centers truncation

_STAGE = os.environ.get("KSTAGE", "full")  # debug bisection: qk|attn|h1|full
_ALLGATHER = os.environ.get("KAG", "1") == "1"  # stats exchange: AllGather vs AllReduce


def _emit(nc, tc, io, es):
    out = io["out"]

    wpool = es.enter_context(tc.tile_pool(name="weights", bufs=1))
    apool = es.enter_context(tc.tile_pool(name="acts", bufs=1))

    # ---------- weight / bias / input loads ----------
    wq_sb = wpool.tile([128, 2, D], F8)
    nc.sync.dma_start(out=wq_sb[:], in_=io["wq8"].rearrange("(c p) o -> p c o", p=128))
    wk_sb = wpool.tile([128, 2, D], F8)
    nc.sync.dma_start(out=wk_sb[:], in_=io["wk8"].rearrange("(c p) o -> p c o", p=128))
    wv_sb = wpool.tile([128, 2, D], F8)
    nc.sync.dma_start(out=wv_sb[:], in_=io["wv8"].rearrange("(c p) o -> p c o", p=128))
    wm_sb = wpool.tile([128, 2, D], BF)
    nc.gpsimd.dma_start(out=wm_sb[:], in_=io["wmT"].rearrange("(c p) o -> p c o", p=128))
    w1x_sb = wpool.tile([128, 2, 2 * D], BF)
    nc.sync.dma_start(out=w1x_sb[:], in_=io["w1xT"].rearrange("(c p) o -> p c o", p=128))
    w1m_sb = wpool.tile([128, 2, 2 * D], BF)
    nc.gpsimd.dma_start(out=w1m_sb[:], in_=io["w1mT"].rearrange("(c p) o -> p c o", p=128))
    w2_sb = wpool.tile([128, 4, D], BF)
    nc.gpsimd.dma_start(out=w2_sb[:], in_=io["w2T"].rearrange("(c p) o -> p c o", p=128))

    bq_sb = wpool.tile([128, 2], FP)
    nc.sync.dma_start(out=bq_sb[:], in_=io["bq"][:])
    bvb_sb = wpool.tile([128, 4, 2, DH], FP)
    nc.sync.dma_start(
        out=bvb_sb[:], in_=io["bvb"].rearrange("p (h r d) -> p h r d", h=4, r=2))
    b1hl_sb = wpool.tile([2, 2 * D], BF)
    nc.sync.dma_start(out=b1hl_sb[:], in_=io["b1hl"][:])
    b2hl_sb = wpool.tile([2, D], BF)
    nc.sync.dma_start(out=b2hl_sb[:], in_=io["b2hl"][:])

    xs8_sb = apool.tile([128, 2, NS], F8)
    nc.sync.dma_start(out=xs8_sb[:], in_=io["xs8"].rearrange("(c p) n -> p c n", p=128))
    xsb_sb = apool.tile([128, 2, NS], BF)
    nc.sync.dma_start(out=xsb_sb[:], in_=io["xsb"].rearrange("(c p) n -> p c n", p=128))
    src8_sb = apool.tile([128, 2, N], F8)
    nc.sync.dma_start(out=src8_sb[:], in_=io["src8"].rearrange("(c p) m -> p c m", p=128))

    ones2 = wpool.tile([2, 512], BF)
    nc.vector.memset(ones2[:], 1.0)
    negc0 = wpool.tile([128, 1], FP)
    nc.vector.memset(negc0[:], -C0)

    # ---------- persistent activation tiles ----------
    qf_sb = apool.tile([128, 2, NS], F8)      # part = 64*hh + d, [kc, n]
    kf_sb = apool.tile([128, 2, N], F8)
    # DoubleRow score layout: one 32-partition tile per (kc, hh), dims [p, r, n]
    q8_t = [[apool.tile([32, 2, NS], F8, name=f"q8_{kc}{hh}") for hh in range(2)]
            for kc in range(2)]
    k8_t = [[apool.tile([32, 2, N], F8, name=f"k8_{kc}{hh}") for hh in range(2)]
            for kc in range(2)]
    # v^T per head + ones col, fp8, stride 80 for DoubleRow
    vaT_sb = apool.tile([128, H, 16, 2, 80], F8)
    exp8_sb = apool.tile([128, 2, JA // 2, 2, 512], F8)   # [., hh, p, r, n]
    prob16_sb = apool.tile([128, 2, JD, 512], I16)        # [., hh, jd, n]
    attn_sb = apool.tile([128, 2, NS], BF)
    msg_sb = apool.tile([128, 2, NS], BF)
    h1_sb = apool.tile([128, 4, NS], FP)
    h1n_sb = apool.tile([128, 4, NS], BF)
    bst_sb = apool.tile([128, 4, 2, 6], FP)
    stats_sb = apool.tile([128, 8], FP)

    nc.vector.memset(vaT_sb[:, :, :, :, DH:DH + 1], 1.0)

    # ---------- phase 1: projections (all fp8 DoubleRow) ----------
    with tc.tile_pool(name="pj", bufs=2, space="PSUM") as pj, \
         tc.tile_pool(name="vt", bufs=2, space="PSUM") as vtp:
        # q: per kc one DoubleRow matmul (contract 256 = 128 part x 2 ic)
        # q/k kc-major with reshuffle right after each kc so scores for the
        # first group (kc=0) can start while kc=1 and v still project.
        for kc in range(2):
            q_ps = pj.tile([128, NS], FP, tag="pj")
            for nh in range(2):
                nc.tensor.matmul(
                    q_ps[:, nh * 512:(nh + 1) * 512],
                    wq_sb[:, :, kc * 128:(kc + 1) * 128],
                    xs8_sb[:, :, nh * 512:(nh + 1) * 512],
                    start=True, stop=True, perf_mode=DR)
            nc.vector.tensor_scalar(
                out=qf_sb[:, kc, :], in0=q_ps[:],
                scalar1=bq_sb[:, kc:kc + 1], scalar2=None, op0=OP.add)
            for mq in range(4):
                k_ps = pj.tile([128, NS], FP, tag="pj")
                for mh in range(2):
                    m0 = mq * NS + mh * 512
                    nc.tensor.matmul(
                        k_ps[:, mh * 512:(mh + 1) * 512],
                        wk_sb[:, :, kc * 128:(kc + 1) * 128],
                        src8_sb[:, :, m0:m0 + 512],
                        start=True, stop=True, perf_mode=DR)
                nc.vector.tensor_copy(kf_sb[:, kc, mq * NS:(mq + 1) * NS], k_ps[:])
            for hh in range(2):
                pi = 64 * hh
                for r in range(2):
                    nc.gpsimd.dma_start(
                        out=q8_t[kc][hh][:, r, :],
                        in_=qf_sb[pi + 32 * r:pi + 32 * r + 32, kc, :])
                    nc.gpsimd.dma_start(
                        out=k8_t[kc][hh][:, r, :],
                        in_=kf_sb[pi + 32 * r:pi + 32 * r + 32, kc, :])
        # v^T: per m-chunk of 128; out[m, (h d)]
        for g in range(8):
            vt_ps = vtp.tile([128, 2, 2, 4, DH], FP, tag="vt")
            for mm in range(4):
                mc = 4 * g + mm
                nc.tensor.matmul(
                    vt_ps[:, mm // 2, mm % 2, :, :],
                    src8_sb[:, :, mc * 128:(mc + 1) * 128], wv_sb[:],
                    start=True, stop=True, perf_mode=DR)
            for pb in range(2):
                nc.vector.tensor_tensor(
                    out=vaT_sb[:, :, 2 * g + pb, :, 0:DH],
                    in0=vt_ps[:, pb, :, :, :].transpose([0, 2, 1, 3]),
                    in1=bvb_sb[:], op=OP.add)

    if _STAGE == "qk":
        o_dbg = apool.tile([128, 2, NS], FP)
        nc.vector.tensor_copy(o_dbg[:, 0, :], qf_sb[:, 0, :])
        nc.vector.tensor_copy(o_dbg[:, 1, :], qf_sb[:, 1, :])
        nc.sync.dma_start(out=out.rearrange("(c p) n -> p c n", p=128), in_=o_dbg[:])
        return

    # ---------- phase 2+3: attention + chunkwise MLP pipeline ----------
    with tc.tile_pool(name="sc", bufs=2, space="PSUM") as scp, \
         tc.tile_pool(name="at", bufs=1, space="PSUM") as atp, \
         tc.tile_pool(name="mm", bufs=2, space="PSUM") as mmp, \
         tc.tile_pool(name="nrm", bufs=2) as nrm:

        def emit_mlp(nch):
            n0 = nch * 512
            # message (bm folded into b1' host-side; 1/Z applied at attn evict)
            for oc in range(2):
                m_ps = mmp.tile([128, 512], FP, tag="mm")
                for ic in range(2):
                    nc.tensor.matmul(
                        m_ps[:], wm_sb[:, ic, oc * 128:(oc + 1) * 128],
                        attn_sb[:, ic, n0:n0 + 512],
                        start=(ic == 0), stop=(ic == 1))
                nc.vector.tensor_copy(msg_sb[:, oc, n0:n0 + 512], m_ps[:])
            # h1 = W1x@x + W1m@msg + b1' (ones-row hi/lo pass)
            for t in range(4):
                h_ps = mmp.tile([128, 512], FP, tag="mm")
                for ic in range(2):
                    nc.tensor.matmul(
                        h_ps[:], w1x_sb[:, ic, t * 128:(t + 1) * 128],
                        xsb_sb[:, ic, n0:n0 + 512], start=(ic == 0), stop=False)
                for ic in range(2):
                    nc.tensor.matmul(
                        h_ps[:], w1m_sb[:, ic, t * 128:(t + 1) * 128],
                        msg_sb[:, ic, n0:n0 + 512], start=False, stop=False)
                nc.tensor.matmul(
                    h_ps[:], b1hl_sb[:, t * 128:(t + 1) * 128], ones2[:],
                    start=False, stop=True)
                nc.vector.tensor_copy(h1_sb[:, t, n0:n0 + 512], h_ps[:])
                nc.vector.bn_stats(out=bst_sb[:, t, nch, :], in_=h1_sb[:, t, n0:n0 + 512])

        for nch in range(2):
            n0 = nch * 512
            for kc in range(2):
                at0 = atp.tile([128, 512], FP, tag="at0")
                at1 = atp.tile([128, 512], FP, tag="at1")
                ats = (at0, at1)
                sched = []  # (engine, pair t, ordinal)
                na = nd = 0
                for t in range(_NPAIR):
                    if t in _DVE_PAIRS:
                        sched.append(("D", t, nd)); nd += 1
                    else:
                        sched.append(("A", t, na)); na += 1
                for eng, t, o in sched:
                    for r in range(2):
                        j = 2 * t + r
                        sc_ps = scp.tile([128, 2, 512], FP, tag="sc")
                        for hh in range(2):
                            nc.tensor.matmul(
                                sc_ps[:, hh, :],
                                k8_t[kc][hh][:, :, j * 128:(j + 1) * 128],
                                q8_t[kc][hh][:, :, n0:n0 + 512],
                                start=True, stop=True, perf_mode=DR)
                        if eng == "A":
                            nc.scalar.activation(
                                out=exp8_sb[:, :, o, r, :], in_=sc_ps[:],
                                func=AF.Exp, scale=0.125, bias=negc0[:])
                        else:
                            nc.vector.tensor_scalar(
                                out=prob16_sb[:, :, 2 * o + r, :], in0=sc_ps[:],
                                scalar1=SCH_A, scalar2=SCH_B, op0=OP.mult, op1=OP.add)
                for hh in range(2):
                    h = 2 * kc + hh
                    at = ats[hh]
                    passes = []  # (lhsT, rhs, perf_mode)
                    for eng, t, o in sched:
                        if eng == "A":
                            passes.append((vaT_sb[:, h, t, :, 0:DH + 1],
                                           exp8_sb[:, hh, o, :, :], DR))
                        else:
                            for r in range(2):
                                passes.append((
                                    vaT_sb[:, h, t, r, 0:DH + 1],
                                    prob16_sb[:, hh, 2 * o + r, :].bitcast(BF),
                                    None))
                    for i, (lhsT, rhs, pm) in enumerate(passes):
                        nc.tensor.matmul(
                            at[:DH + 1, :], lhsT, rhs,
                            start=(i == 0), stop=(i == len(passes) - 1),
                            perf_mode=pm)
                for hh in range(2):
                    at = ats[hh]
                    rz = nrm.tile([1, 512], FP, tag="rz")
                    nc.vector.reciprocal(rz[:], at[DH:DH + 1, :])
                    rzb = nrm.tile([DH, 512], FP, tag="rzb")
                    nc.gpsimd.partition_broadcast(rzb[:], rz[:])
                    nc.vector.tensor_mul(
                        attn_sb[64 * hh:64 * hh + DH, kc, n0:n0 + 512],
                        at[0:DH, :], rzb[:])
            emit_mlp(nch)

        if _STAGE == "attn":
            o_dbg = apool.tile([128, 2, NS], FP)
            nc.vector.tensor_copy(o_dbg[:, 0, :], attn_sb[:, 0, :])
            nc.vector.tensor_copy(o_dbg[:, 1, :], attn_sb[:, 1, :])
            nc.sync.dma_start(out=out.rearrange("(c p) n -> p c n", p=128), in_=o_dbg[:])
            return
        if _STAGE == "h1":
            o_dbg = apool.tile([128, 2, NS], FP)
            nc.vector.tensor_copy(o_dbg[:, 0, :], h1_sb[:, 0, :])
            nc.vector.tensor_copy(o_dbg[:, 1, :], h1_sb[:, 1, :])
            nc.sync.dma_start(out=out.rearrange("(c p) n -> p c n", p=128), in_=o_dbg[:])
            return

        # ---------- instance-norm stats exchange + output ----------
        with tc.tile_pool(name="dram", bufs=1, space="DRAM") as dram, \
             tc.tile_pool(name="nstat", bufs=1) as nstat:
            # local (sum, sumsq) per channel from bn stats
            for t in range(4):
                mv = nstat.tile([128, 2], FP, tag="mv")
                nc.vector.bn_aggr(out=mv[:], in_=bst_sb[:, t, :, :])
                nc.vector.tensor_scalar_mul(stats_sb[:, t:t + 1], mv[:, 0:1], float(NS))
                msq = nstat.tile([128, 1], FP, tag="msq")
                nc.vector.tensor_mul(msq[:], mv[:, 0:1], mv[:, 0:1])
                msq2 = nstat.tile([128, 1], FP, tag="msq2")
                nc.vector.tensor_add(msq2[:], mv[:, 1:2], msq[:])
                nc.vector.tensor_scalar_mul(stats_sb[:, 4 + t:5 + t], msq2[:], float(NS))

            cc_in = dram.tile([128, 8], FP)
            if _ALLGATHER:
                cc_out = dram.tile([4, 128, 8], FP)
                nc.sync.dma_start(out=cc_in[:], in_=stats_sb[:])
                nc.gpsimd.collective_compute(
                    "AllGather", OP.bypass,
                    replica_groups=[[0, 1, 2, 3], [4, 5, 6, 7]],
                    ins=[cc_in[:].opt()], outs=[cc_out[:].opt()],
                )
                sred4 = nstat.tile([128, 4, 8], FP)
                nc.sync.dma_start(out=sred4[:], in_=cc_out.rearrange("k p s -> p k s"))
                s01 = nstat.tile([128, 8], FP)
                nc.vector.tensor_add(s01[:], sred4[:, 0, :], sred4[:, 1, :])
                s23 = nstat.tile([128, 8], FP)
                nc.vector.tensor_add(s23[:], sred4[:, 2, :], sred4[:, 3, :])
                sred = nstat.tile([128, 8], FP)
                nc.vector.tensor_add(sred[:], s01[:], s23[:])
            else:
                cc_out = dram.tile([128, 8], FP)
                nc.sync.dma_start(out=cc_in[:], in_=stats_sb[:])
                nc.gpsimd.collective_compute(
                    "AllReduce", OP.add,
                    replica_groups=[[0, 1, 2, 3], [4, 5, 6, 7]],
                    ins=[cc_in[:].opt()], outs=[cc_out[:].opt()],
                )
                sred = nstat.tile([128, 8], FP)
                nc.sync.dma_start(out=sred[:], in_=cc_out[:])

            mu4 = nstat.tile([128, 4], FP)
            nc.vector.tensor_scalar_mul(mu4[:], sred[:, 0:4], 1.0 / N)
            e24 = nstat.tile([128, 4], FP)
            nc.vector.tensor_scalar_mul(e24[:], sred[:, 4:8], 1.0 / N)
            var4 = nstat.tile([128, 4], FP)
            nc.vector.tensor_mul(var4[:], mu4[:], mu4[:])
            nc.vector.tensor_tensor(out=var4[:], in0=e24[:], in1=var4[:], op=OP.subtract)
            eps1 = nstat.tile([128, 1], FP)
            nc.vector.memset(eps1[:], EPS)
            std4 = nstat.tile([128, 4], FP)
            nc.scalar.activation(out=std4[:], in_=var4[:], func=AF.Sqrt, bias=eps1[:])
            rstd4 = nstat.tile([128, 4], FP)
            nc.vector.reciprocal(rstd4[:], std4[:])
            nb4 = nstat.tile([128, 4], FP)
            nc.vector.tensor_mul(nb4[:], mu4[:], rstd4[:])
            nc.vector.tensor_scalar_mul(nb4[:], nb4[:], -1.0)

            # h = relu(h1 * rstd - mu * rstd)
            for t in range(4):
                nc.scalar.activation(
                    out=h1n_sb[:, t, :], in_=h1_sb[:, t, :], func=AF.Relu,
                    bias=nb4[:, t:t + 1], scale=rstd4[:, t:t + 1])

            # out = W2T.T @ h + b2 (ones-row hi/lo pass)
            out_sb = apool.tile([128, 2, NS], FP)
            outp = out.rearrange("(c p) n -> p c n", p=128)
            for oc in range(2):
                for nch in range(2):
                    n0 = nch * 512
                    o_ps = mmp.tile([128, 512], FP, tag="mm")
                    for kc2 in range(4):
                        nc.tensor.matmul(
                            o_ps[:], w2_sb[:, kc2, oc * 128:(oc + 1) * 128],
                            h1n_sb[:, kc2, n0:n0 + 512],
                            start=(kc2 == 0), stop=False)
                    nc.tensor.matmul(
                        o_ps[:], b2hl_sb[:, oc * 128:(oc + 1) * 128], ones2[:],
                        start=False, stop=True)
                    nc.vector.tensor_copy(out_sb[:, oc, n0:n0 + 512], o_ps[:])
                    nc.sync.dma_start(out=outp[:, oc, n0:n0 + 512], in_=out_sb[:, oc, n0:n0 + 512])


_BUILT = {}


def _build():
    if "nc" in _BUILT:
        return _BUILT["nc"]
    nc = bacc.Bacc("TRN2", target_bir_lowering=False, debug=False,
                   enable_asserts=True, num_devices=NCORES)
    io = {}
    io["xs8"] = nc.dram_tensor("xs8", [D, NS], F8, kind="ExternalInput").ap()
    io["xsb"] = nc.dram_tensor("xsb", [D, NS], BF, kind="ExternalInput").ap()
    io["src8"] = nc.dram_tensor("src8", [D, N], F8, kind="ExternalInput").ap()
    io["wq8"] = nc.dram_tensor("wq8", [D, D], F8, kind="ExternalInput").ap()
    io["wk8"] = nc.dram_tensor("wk8", [D, D], F8, kind="ExternalInput").ap()
    io["wv8"] = nc.dram_tensor("wv8", [D, D], F8, kind="ExternalInput").ap()
    io["wmT"] = nc.dram_tensor("wmT", [D, D], BF, kind="ExternalInput").ap()
    io["w1xT"] = nc.dram_tensor("w1xT", [D, 2 * D], BF, kind="ExternalInput").ap()
    io["w1mT"] = nc.dram_tensor("w1mT", [D, 2 * D], BF, kind="ExternalInput").ap()
    io["w2T"] = nc.dram_tensor("w2T", [2 * D, D], BF, kind="ExternalInput").ap()
    io["bq"] = nc.dram_tensor("bq", [128, 2], FP, kind="ExternalInput").ap()
    io["bvb"] = nc.dram_tensor("bvb", [128, 8 * DH], FP, kind="ExternalInput").ap()
    io["b1hl"] = nc.dram_tensor("b1hl", [2, 2 * D], BF, kind="ExternalInput").ap()
    io["b2hl"] = nc.dram_tensor("b2hl", [2, D], BF, kind="ExternalInput").ap()
    io["out"] = nc.dram_tensor("out", [D, NS], FP, kind="ExternalOutput").ap()

    import contextlib
    with tile.TileContext(nc) as tc:
        with contextlib.ExitStack() as es:
            _emit(nc, tc, io, es)
    nc.compile()
    _BUILT["nc"] = nc
    return nc


def _prep_inputs(x, source, Wq, bq, Wk, bk, Wv, bv, Wm, bm, W1, b1, W2, b2):
    import ml_dtypes
    npF8 = mybir.dt.np(F8)
    npBF = ml_dtypes.bfloat16
    perm = np.array([4 * d + h for h in range(H) for d in range(DH)])
    f32 = lambda a: np.ascontiguousarray(a, dtype=np.float32)
    bf = lambda a: np.ascontiguousarray(np.asarray(a, np.float32), dtype=npBF)
    f8c = lambda a: np.ascontiguousarray(np.asarray(a, np.float32), dtype=npF8)

    b1p = np.asarray(b1, np.float64) + np.asarray(W1, np.float64)[:, D:] @ np.asarray(bm, np.float64)
    b1hi = np.asarray(b1p, np.float32).astype(npBF)
    b1lo = (np.asarray(b1p, np.float32) - b1hi.astype(np.float32)).astype(npBF)
    b2hi = np.asarray(b2, np.float32).astype(npBF)
    b2lo = (np.asarray(b2, np.float32) - b2hi.astype(np.float32)).astype(npBF)

    bvp = np.asarray(bv, np.float32)[perm]                    # (h, d) order
    bvb = np.tile(bvp.reshape(4, 1, DH), (1, 2, 1)).reshape(1, 8 * DH)
    bvb = np.ascontiguousarray(np.broadcast_to(bvb, (128, 8 * DH)), np.float32)

    shared = {
        "wq8": f8c(Wq[perm, :].T),
        "wk8": f8c(Wk[perm, :].T),
        "wv8": f8c(Wv[perm, :].T),
        "wmT": bf(Wm[:, perm].T),
        "w1xT": bf(W1.T[0:D, :]),
        "w1mT": bf(W1.T[D:2 * D, :]),
        "w2T": bf(W2.T),
        "bq": f32(bq[perm].reshape(2, 128).T),
        "bvb": bvb,
        "b1hl": np.ascontiguousarray(np.stack([b1hi, b1lo])),
        "b2hl": np.ascontiguousarray(np.stack([b2hi, b2lo])),
    }
    in_maps = []
    for core in range(NCORES):
        b, s = core // 4, core % 4
        m = dict(shared)
        xs = x[b][:, s * NS:(s + 1) * NS]
        m["xs8"] = f8c(xs)
        m["xsb"] = bf(xs)
        m["src8"] = f8c(source[b])
        in_maps.append(m)
    return in_maps


def run(inputs, **spmd_kwargs):
    """Build (cached), run on cores 0-7, return (full_output, BassKernelResults)."""
    nc = _build()
    in_maps = _prep_inputs(**inputs)
    res = bass_utils.run_bass_kernel_spmd(
        nc, in_maps, core_ids=list(range(NCORES)), **spmd_kwargs)
    full = np.empty((B, D, N), dtype=np.float32)
    for core in range(NCORES):
        b, s = core // 4, core % 4
        full[b][:, s * NS:(s + 1) * NS] = res.results[core]["out"]
    return full, res


def kernel(**inputs):
    full, _ = run(inputs)
    return full


# revision 28
# speedup vs baseline: 1.4192x; 1.0626x over previous
# Trainium2 Bass kernel for nn_AttentionalPropagation (B=2, D=256, N=M=4096, H=4).
#
# Sharding: 8 cores; each batch (B=2) owns 4 cores; each core computes a
# 1024-column sequence shard of the output end-to-end. k,v are computed
# redundantly per core from the full `source` of its batch. Cross-core
# communication: one AllGather of InstanceNorm partial (sum, sumsq) stats
# within each 4-core batch group (+ local sum), cheaper in the cost model
# than AllReduce.
#
# Engine plan (per core, cost-model driven):
#  - PE: all projections as fp8 DoubleRow (host-side fp8 conversion of
#    x/source/Wq/Wk/Wv), fp8 DoubleRow scores, mixed fp8-DoubleRow/bf16
#    attention, bf16 msg/h1/out. Biases enter via DVE evictions (q, v),
#    hi/lo bf16 ones-row matmul passes (b1', b2), or cancel entirely
#    (bk shifts each softmax column by a constant over m -> dropped;
#    bm is folded into b1' host-side).
#  - Softmax: scores/8 - 1 (offset keeps fp8 exp in range; cancels in the
#    normalization). exp is split between ACT (fp8 output, feeds DoubleRow
#    attn passes) and DVE (Schraudolph int16 bit-trick -> bf16 probs, feeds
#    plain bf16 attn passes). Denominator via the ones-column in v^T.
#  - Evictions that need no arithmetic go over DMA queues (msg, h1, out).

import os

import numpy as np

import concourse.bass as bass  # noqa: F401
import concourse.tile as tile
import concourse.mybir as mybir
from concourse import bacc
from concourse import bass_utils

B, D, N = 2, 256, 4096
H, DH = 4, 64
NS = N // 4           # sequence shard per core
NCORES = 8
EPS = 1e-5

FP = mybir.dt.float32
BF = mybir.dt.bfloat16
F8 = mybir.dt.float8e4
I16 = mybir.dt.int16
OP = mybir.AluOpType
AF = mybir.ActivationFunctionType
DR = mybir.MatmulPerfMode.DoubleRow

JA = 20               # exp chunks per group on ACT (fp8, DoubleRow attn)
JD = 32 - JA          # exp chunks per group on DVE (Schraudolph bf16)
# Interleave ACT/DVE ownership over the 16 m-chunk PAIRS of a group so both
# engines run concurrently (ACT pairs feed DoubleRow attn; DVE pairs bf16).
_NPAIR, _NDP = 16, JD // 2
_DVE_PAIRS = sorted({int(round((i + 0.5) * _NPAIR / _NDP - 0.5)) for i in range(_NDP)})
assert len(_DVE_PAIRS) == _NDP
C0 = 1.0              # exp offset: probs = exp(s/8 - C0); cancels in softmax
LOG2E = 1.4426950408889634
SCH_A = 0.125 * 128 * LOG2E                       # i16 = s*SCH_A + SCH_B
SCH_B = 127.0 * 128 - 128 * C0 * LOG2E - 0.5      # -0.5 centers truncation

_STAGE = os.environ.get("KSTAGE", "full")  # debug bisection: qk|attn|h1|full
_ALLGATHER = os.environ.get("KAG", "1") == "1"  # stats exchange: AllGather vs AllReduce


def _emit(nc, tc, io, es):
    out = io["out"]

    wpool = es.enter_context(tc.tile_pool(name="weights", bufs=1))
    apool = es.enter_context(tc.tile_pool(name="acts", bufs=1))

    # ---------- weight / bias / input loads ----------
    wq_sb = wpool.tile([128, 2, D], F8)
    nc.sync.dma_start(out=wq_sb[:], in_=io["wq8"].rearrange("(c p) o -> p c o", p=128))
    wk_sb = wpool.tile([128, 2, D], F8)
    nc.sync.dma_start(out=wk_sb[:], in_=io["wk8"].rearrange("(c p) o -> p c o", p=128))
    wv_sb = wpool.tile([128, 2, D], F8)
    nc.sync.dma_start(out=wv_sb[:], in_=io["wv8"].rearrange("(c p) o -> p c o", p=128))
    wm_sb = wpool.tile([128, 2, D], BF)
    nc.gpsimd.dma_start(out=wm_sb[:], in_=io["wmT"].rearrange("(c p) o -> p c o", p=128))
    w1x_sb = wpool.tile([128, 2, 2 * D], BF)
    nc.sync.dma_start(out=w1x_sb[:], in_=io["w1xT"].rearrange("(c p) o -> p c o", p=128))
    w1m_sb = wpool.tile([128, 2, 2 * D], BF)
    nc.gpsimd.dma_start(out=w1m_sb[:], in_=io["w1mT"].rearrange("(c p) o -> p c o", p=128))
    w2_sb = wpool.tile([128, 4, D], BF)
    nc.gpsimd.dma_start(out=w2_sb[:], in_=io["w2T"].rearrange("(c p) o -> p c o", p=128))

    bq_sb = wpool.tile([128, 2], FP)
    nc.sync.dma_start(out=bq_sb[:], in_=io["bq"][:])
    bvb_sb = wpool.tile([128, 4, 2, DH], FP)
    nc.sync.dma_start(
        out=bvb_sb[:], in_=io["bvb"].rearrange("p (h r d) -> p h r d", h=4, r=2))
    b1hl_sb = wpool.tile([2, 2 * D], BF)
    nc.sync.dma_start(out=b1hl_sb[:], in_=io["b1hl"][:])
    b2hl_sb = wpool.tile([2, D], BF)
    nc.sync.dma_start(out=b2hl_sb[:], in_=io["b2hl"][:])

    xs8_sb = apool.tile([128, 2, NS], F8)
    nc.sync.dma_start(out=xs8_sb[:], in_=io["xs8"].rearrange("(c p) n -> p c n", p=128))
    xsb_sb = apool.tile([128, 2, NS], BF)
    nc.sync.dma_start(out=xsb_sb[:], in_=io["xsb"].rearrange("(c p) n -> p c n", p=128))
    src8_sb = apool.tile([128, 2, N], F8)
    nc.sync.dma_start(out=src8_sb[:], in_=io["src8"].rearrange("(c p) m -> p c m", p=128))

    ones2 = wpool.tile([2, 512], BF)
    nc.vector.memset(ones2[:], 1.0)
    negc0 = wpool.tile([128, 1], FP)
    nc.vector.memset(negc0[:], -C0)

    # ---------- persistent activation tiles ----------
    qf_sb = apool.tile([128, 2, NS], F8)      # part = 64*hh + d, [kc, n]
    kf_sb = apool.tile([128, 2, N], F8)
    # DoubleRow score layout: one 32-partition tile per (kc, hh), dims [p, r, n]
    q8_t = [[apool.tile([32, 2, NS], F8, name=f"q8_{kc}{hh}") for hh in range(2)]
            for kc in range(2)]
    k8_t = [[apool.tile([32, 2, N], F8, name=f"k8_{kc}{hh}") for hh in range(2)]
            for kc in range(2)]
    # v^T per head + ones col, fp8, stride 80 for DoubleRow
    vaT_sb = apool.tile([128, H, 16, 2, 80], F8)
    exp8_sb = apool.tile([128, 2, JA // 2, 2, 512], F8)   # [., hh, p, r, n]
    prob16_sb = apool.tile([128, 2, JD, 512], I16)        # [., hh, jd, n]
    attn_sb = apool.tile([128, 2, NS], BF)
    msg_sb = apool.tile([128, 2, NS], BF)
    h1_sb = apool.tile([128, 4, NS], FP)
    h1n_sb = apool.tile([128, 4, NS], BF)
    bst_sb = apool.tile([128, 4, 2, 6], FP)
    stats_sb = apool.tile([128, 8], FP)

    nc.vector.memset(vaT_sb[:, :, :, :, DH:DH + 1], 1.0)

    # ---------- phase 1: projections (all fp8 DoubleRow) ----------
    with tc.tile_pool(name="pj", bufs=2, space="PSUM") as pj, \
         tc.tile_pool(name="vt", bufs=2, space="PSUM") as vtp:
        # q: per kc one DoubleRow matmul (contract 256 = 128 part x 2 ic)
        # q/k kc-major with reshuffle right after each kc so scores for the
        # first group (kc=0) can start while kc=1 and v still project.
        for kc in range(2):
            q_ps = pj.tile([128, NS], FP, tag="pj")
            for nh in range(2):
                nc.tensor.matmul(
                    q_ps[:, nh * 512:(nh + 1) * 512],
                    wq_sb[:, :, kc * 128:(kc + 1) * 128],
                    xs8_sb[:, :, nh * 512:(nh + 1) * 512],
                    start=True, stop=True, perf_mode=DR)
            nc.vector.tensor_scalar(
                out=qf_sb[:, kc, :], in0=q_ps[:],
                scalar1=bq_sb[:, kc:kc + 1], scalar2=None, op0=OP.add)
            for mq in range(4):
                k_ps = pj.tile([128, NS], FP, tag="pj")
                for mh in range(2):
                    m0 = mq * NS + mh * 512
                    nc.tensor.matmul(
                        k_ps[:, mh * 512:(mh + 1) * 512],
                        wk_sb[:, :, kc * 128:(kc + 1) * 128],
                        src8_sb[:, :, m0:m0 + 512],
                        start=True, stop=True, perf_mode=DR)
                nc.scalar.copy(kf_sb[:, kc, mq * NS:(mq + 1) * NS], k_ps[:])
            for hh in range(2):
                pi = 64 * hh
                for r in range(2):
                    nc.gpsimd.dma_start(
                        out=q8_t[kc][hh][:, r, :],
                        in_=qf_sb[pi + 32 * r:pi + 32 * r + 32, kc, :])
                    nc.gpsimd.dma_start(
                        out=k8_t[kc][hh][:, r, :],
                        in_=kf_sb[pi + 32 * r:pi + 32 * r + 32, kc, :])
        # v^T: per m-chunk of 128; out[m, (h d)]
        for g in range(8):
            vt_ps = vtp.tile([128, 2, 2, 4, DH], FP, tag="vt")
            for mm in range(4):
                mc = 4 * g + mm
                nc.tensor.matmul(
                    vt_ps[:, mm // 2, mm % 2, :, :],
                    src8_sb[:, :, mc * 128:(mc + 1) * 128], wv_sb[:],
                    start=True, stop=True, perf_mode=DR)
            for pb in range(2):
                nc.vector.tensor_tensor(
                    out=vaT_sb[:, :, 2 * g + pb, :, 0:DH],
                    in0=vt_ps[:, pb, :, :, :].transpose([0, 2, 1, 3]),
                    in1=bvb_sb[:], op=OP.add)

    if _STAGE == "qk":
        o_dbg = apool.tile([128, 2, NS], FP)
        nc.vector.tensor_copy(o_dbg[:, 0, :], qf_sb[:, 0, :])
        nc.vector.tensor_copy(o_dbg[:, 1, :], qf_sb[:, 1, :])
        nc.sync.dma_start(out=out.rearrange("(c p) n -> p c n", p=128), in_=o_dbg[:])
        return

    # ---------- phase 2+3: attention + chunkwise MLP pipeline ----------
    with tc.tile_pool(name="sc", bufs=2, space="PSUM") as scp, \
         tc.tile_pool(name="at", bufs=1, space="PSUM") as atp, \
         tc.tile_pool(name="mm", bufs=2, space="PSUM") as mmp, \
         tc.tile_pool(name="nrm", bufs=2) as nrm:

        def emit_mlp(nch):
            n0 = nch * 512
            # message (bm folded into b1' host-side; 1/Z applied at attn evict)
            for oc in range(2):
                m_ps = mmp.tile([128, 512], FP, tag="mm")
                for ic in range(2):
                    nc.tensor.matmul(
                        m_ps[:], wm_sb[:, ic, oc * 128:(oc + 1) * 128],
                        attn_sb[:, ic, n0:n0 + 512],
                        start=(ic == 0), stop=(ic == 1))
                nc.vector.tensor_copy(msg_sb[:, oc, n0:n0 + 512], m_ps[:])
            # h1 = W1x@x + W1m@msg + b1' (ones-row hi/lo pass)
            for t in range(4):
                h_ps = mmp.tile([128, 512], FP, tag="mm")
                for ic in range(2):
                    nc.tensor.matmul(
                        h_ps[:], w1x_sb[:, ic, t * 128:(t + 1) * 128],
                        xsb_sb[:, ic, n0:n0 + 512], start=(ic == 0), stop=False)
                for ic in range(2):
                    nc.tensor.matmul(
                        h_ps[:], w1m_sb[:, ic, t * 128:(t + 1) * 128],
                        msg_sb[:, ic, n0:n0 + 512], start=False, stop=False)
                nc.tensor.matmul(
                    h_ps[:], b1hl_sb[:, t * 128:(t + 1) * 128], ones2[:],
                    start=False, stop=True)
                nc.vector.tensor_copy(h1_sb[:, t, n0:n0 + 512], h_ps[:])
                nc.vector.bn_stats(out=bst_sb[:, t, nch, :], in_=h1_sb[:, t, n0:n0 + 512])

        for nch in range(2):
            n0 = nch * 512
            for kc in range(2):
                at0 = atp.tile([128, 512], FP, tag="at0")
                at1 = atp.tile([128, 512], FP, tag="at1")
                ats = (at0, at1)
                sched = []  # (engine, pair t, ordinal)
                na = nd = 0
                for t in range(_NPAIR):
                    if t in _DVE_PAIRS:
                        sched.append(("D", t, nd)); nd += 1
                    else:
                        sched.append(("A", t, na)); na += 1
                # chunk-level emission order: merge the two j-streams so ACT
                # stays saturated while DVE consumes concurrently (2 PSUM bufs)
                a_js = [(2 * t + r, o, r) for e, t, o in sched if e == "A" for r in range(2)]
                d_js = [(2 * t + r, o, r) for e, t, o in sched if e == "D" for r in range(2)]
                order = []
                ca = cd = 0
                for _ in range(2 * _NPAIR):
                    if cd * 2 * JA <= ca * 2 * JD and cd < len(d_js):
                        order.append(("D",) + d_js[cd]); cd += 1
                    elif ca < len(a_js):
                        order.append(("A",) + a_js[ca]); ca += 1
                    else:
                        order.append(("D",) + d_js[cd]); cd += 1
                for eng, j, o, r in order:
                    sc_ps = scp.tile([128, 2, 512], FP, tag="sc")
                    for hh in range(2):
                        nc.tensor.matmul(
                            sc_ps[:, hh, :],
                            k8_t[kc][hh][:, :, j * 128:(j + 1) * 128],
                            q8_t[kc][hh][:, :, n0:n0 + 512],
                            start=True, stop=True, perf_mode=DR)
                    if eng == "A":
                        nc.scalar.activation(
                            out=exp8_sb[:, :, o, r, :], in_=sc_ps[:],
                            func=AF.Exp, scale=0.125, bias=negc0[:])
                    else:
                        nc.vector.tensor_scalar(
                            out=prob16_sb[:, :, 2 * o + r, :], in0=sc_ps[:],
                            scalar1=SCH_A, scalar2=SCH_B, op0=OP.mult, op1=OP.add)
                for hh in range(2):
                    h = 2 * kc + hh
                    at = ats[hh]
                    passes = []  # (lhsT, rhs, perf_mode)
                    for eng, t, o in sched:
                        if eng == "A":
                            passes.append((vaT_sb[:, h, t, :, 0:DH + 1],
                                           exp8_sb[:, hh, o, :, :], DR))
                        else:
                            for r in range(2):
                                passes.append((
                                    vaT_sb[:, h, t, r, 0:DH + 1],
                                    prob16_sb[:, hh, 2 * o + r, :].bitcast(BF),
                                    None))
                    for i, (lhsT, rhs, pm) in enumerate(passes):
                        nc.tensor.matmul(
                            at[:DH + 1, :], lhsT, rhs,
                            start=(i == 0), stop=(i == len(passes) - 1),
                            perf_mode=pm)
                for hh in range(2):
                    at = ats[hh]
                    rz = nrm.tile([1, 512], FP, tag="rz")
                    nc.vector.reciprocal(rz[:], at[DH:DH + 1, :])
                    rzb = nrm.tile([DH, 512], FP, tag="rzb")
                    nc.gpsimd.partition_broadcast(rzb[:], rz[:])
                    nc.vector.tensor_mul(
                        attn_sb[64 * hh:64 * hh + DH, kc, n0:n0 + 512],
                        at[0:DH, :], rzb[:])
            emit_mlp(nch)

        if _STAGE == "attn":
            o_dbg = apool.tile([128, 2, NS], FP)
            nc.vector.tensor_copy(o_dbg[:, 0, :], attn_sb[:, 0, :])
            nc.vector.tensor_copy(o_dbg[:, 1, :], attn_sb[:, 1, :])
            nc.sync.dma_start(out=out.rearrange("(c p) n -> p c n", p=128), in_=o_dbg[:])
            return
        if _STAGE == "h1":
            o_dbg = apool.tile([128, 2, NS], FP)
            nc.vector.tensor_copy(o_dbg[:, 0, :], h1_sb[:, 0, :])
            nc.vector.tensor_copy(o_dbg[:, 1, :], h1_sb[:, 1, :])
            nc.sync.dma_start(out=out.rearrange("(c p) n -> p c n", p=128), in_=o_dbg[:])
            return

        # ---------- instance-norm stats exchange + output ----------
        with tc.tile_pool(name="dram", bufs=1, space="DRAM") as dram, \
             tc.tile_pool(name="nstat", bufs=1) as nstat:
            # local (sum, sumsq) per channel from bn stats
            for t in range(4):
                mv = nstat.tile([128, 2], FP, tag="mv")
                nc.vector.bn_aggr(out=mv[:], in_=bst_sb[:, t, :, :])
                nc.vector.tensor_scalar_mul(stats_sb[:, t:t + 1], mv[:, 0:1], float(NS))
                msq = nstat.tile([128, 1], FP, tag="msq")
                nc.vector.tensor_mul(msq[:], mv[:, 0:1], mv[:, 0:1])
                msq2 = nstat.tile([128, 1], FP, tag="msq2")
                nc.vector.tensor_add(msq2[:], mv[:, 1:2], msq[:])
                nc.vector.tensor_scalar_mul(stats_sb[:, 4 + t:5 + t], msq2[:], float(NS))

            cc_in = dram.tile([128, 8], FP)
            if _ALLGATHER:
                cc_out = dram.tile([4, 128, 8], FP)
                nc.sync.dma_start(out=cc_in[:], in_=stats_sb[:])
                nc.gpsimd.collective_compute(
                    "AllGather", OP.bypass,
                    replica_groups=[[0, 1, 2, 3], [4, 5, 6, 7]],
                    ins=[cc_in[:].opt()], outs=[cc_out[:].opt()],
                )
                sred4 = nstat.tile([128, 4, 8], FP)
                nc.sync.dma_start(out=sred4[:], in_=cc_out.rearrange("k p s -> p k s"))
                s01 = nstat.tile([128, 8], FP)
                nc.vector.tensor_add(s01[:], sred4[:, 0, :], sred4[:, 1, :])
                s23 = nstat.tile([128, 8], FP)
                nc.vector.tensor_add(s23[:], sred4[:, 2, :], sred4[:, 3, :])
                sred = nstat.tile([128, 8], FP)
                nc.vector.tensor_add(sred[:], s01[:], s23[:])
            else:
                cc_out = dram.tile([128, 8], FP)
                nc.sync.dma_start(out=cc_in[:], in_=stats_sb[:])
                nc.gpsimd.collective_compute(
                    "AllReduce", OP.add,
                    replica_groups=[[0, 1, 2, 3], [4, 5, 6, 7]],
                    ins=[cc_in[:].opt()], outs=[cc_out[:].opt()],
                )
                sred = nstat.tile([128, 8], FP)
                nc.sync.dma_start(out=sred[:], in_=cc_out[:])

            mu4 = nstat.tile([128, 4], FP)
            nc.vector.tensor_scalar_mul(mu4[:], sred[:, 0:4], 1.0 / N)
            e24 = nstat.tile([128, 4], FP)
            nc.vector.tensor_scalar_mul(e24[:], sred[:, 4:8], 1.0 / N)
            var4 = nstat.tile([128, 4], FP)
            nc.vector.tensor_mul(var4[:], mu4[:], mu4[:])
            nc.vector.tensor_tensor(out=var4[:], in0=e24[:], in1=var4[:], op=OP.subtract)
            eps1 = nstat.tile([128, 1], FP)
            nc.vector.memset(eps1[:], EPS)
            std4 = nstat.tile([128, 4], FP)
            nc.scalar.activation(out=std4[:], in_=var4[:], func=AF.Sqrt, bias=eps1[:])
            rstd4 = nstat.tile([128, 4], FP)
            nc.vector.reciprocal(rstd4[:], std4[:])
            nb4 = nstat.tile([128, 4], FP)
            nc.vector.tensor_mul(nb4[:], mu4[:], rstd4[:])
            nc.vector.tensor_scalar_mul(nb4[:], nb4[:], -1.0)

            # h = relu(h1 * rstd - mu * rstd)
            for t in range(4):
                nc.scalar.activation(
                    out=h1n_sb[:, t, :], in_=h1_sb[:, t, :], func=AF.Relu,
                    bias=nb4[:, t:t + 1], scale=rstd4[:, t:t + 1])

            # out = W2T.T @ h + b2 (ones-row hi/lo pass)
            out_sb = apool.tile([128, 2, NS], FP)
            outp = out.rearrange("(c p) n -> p c n", p=128)
            for oc in range(2):
                for nch in range(2):
                    n0 = nch * 512
                    o_ps = mmp.tile([128, 512], FP, tag="mm")
                    for kc2 in range(4):
                        nc.tensor.matmul(
                            o_ps[:], w2_sb[:, kc2, oc * 128:(oc + 1) * 128],
                            h1n_sb[:, kc2, n0:n0 + 512],
                            start=(kc2 == 0), stop=False)
                    nc.tensor.matmul(
                        o_ps[:], b2hl_sb[:, oc * 128:(oc + 1) * 128], ones2[:],
                        start=False, stop=True)
                    nc.vector.tensor_copy(out_sb[:, oc, n0:n0 + 512], o_ps[:])
                    nc.sync.dma_start(out=outp[:, oc, n0:n0 + 512], in_=out_sb[:, oc, n0:n0 + 512])


_BUILT = {}


def _build():
    if "nc" in _BUILT:
        return _BUILT["nc"]
    nc = bacc.Bacc("TRN2", target_bir_lowering=False, debug=False,
                   enable_asserts=True, num_devices=NCORES)
    io = {}
    io["xs8"] = nc.dram_tensor("xs8", [D, NS], F8, kind="ExternalInput").ap()
    io["xsb"] = nc.dram_tensor("xsb", [D, NS], BF, kind="ExternalInput").ap()
    io["src8"] = nc.dram_tensor("src8", [D, N], F8, kind="ExternalInput").ap()
    io["wq8"] = nc.dram_tensor("wq8", [D, D], F8, kind="ExternalInput").ap()
    io["wk8"] = nc.dram_tensor("wk8", [D, D], F8, kind="ExternalInput").ap()
    io["wv8"] = nc.dram_tensor("wv8", [D, D], F8, kind="ExternalInput").ap()
    io["wmT"] = nc.dram_tensor("wmT", [D, D], BF, kind="ExternalInput").ap()
    io["w1xT"] = nc.dram_tensor("w1xT", [D, 2 * D], BF, kind="ExternalInput").ap()
    io["w1mT"] = nc.dram_tensor("w1mT", [D, 2 * D], BF, kind="ExternalInput").ap()
    io["w2T"] = nc.dram_tensor("w2T", [2 * D, D], BF, kind="ExternalInput").ap()
    io["bq"] = nc.dram_tensor("bq", [128, 2], FP, kind="ExternalInput").ap()
    io["bvb"] = nc.dram_tensor("bvb", [128, 8 * DH], FP, kind="ExternalInput").ap()
    io["b1hl"] = nc.dram_tensor("b1hl", [2, 2 * D], BF, kind="ExternalInput").ap()
    io["b2hl"] = nc.dram_tensor("b2hl", [2, D], BF, kind="ExternalInput").ap()
    io["out"] = nc.dram_tensor("out", [D, NS], FP, kind="ExternalOutput").ap()

    import contextlib
    with tile.TileContext(nc) as tc:
        with contextlib.ExitStack() as es:
            _emit(nc, tc, io, es)
    nc.compile()
    _BUILT["nc"] = nc
    return nc


def _prep_inputs(x, source, Wq, bq, Wk, bk, Wv, bv, Wm, bm, W1, b1, W2, b2):
    import ml_dtypes
    npF8 = mybir.dt.np(F8)
    npBF = ml_dtypes.bfloat16
    perm = np.array([4 * d + h for h in range(H) for d in range(DH)])
    f32 = lambda a: np.ascontiguousarray(a, dtype=np.float32)
    bf = lambda a: np.ascontiguousarray(np.asarray(a, np.float32), dtype=npBF)
    f8c = lambda a: np.ascontiguousarray(np.asarray(a, np.float32), dtype=npF8)

    b1p = np.asarray(b1, np.float64) + np.asarray(W1, np.float64)[:, D:] @ np.asarray(bm, np.float64)
    b1hi = np.asarray(b1p, np.float32).astype(npBF)
    b1lo = (np.asarray(b1p, np.float32) - b1hi.astype(np.float32)).astype(npBF)
    b2hi = np.asarray(b2, np.float32).astype(npBF)
    b2lo = (np.asarray(b2, np.float32) - b2hi.astype(np.float32)).astype(npBF)

    bvp = np.asarray(bv, np.float32)[perm]                    # (h, d) order
    bvb = np.tile(bvp.reshape(4, 1, DH), (1, 2, 1)).reshape(1, 8 * DH)
    bvb = np.ascontiguousarray(np.broadcast_to(bvb, (128, 8 * DH)), np.float32)

    shared = {
        "wq8": f8c(Wq[perm, :].T),
        "wk8": f8c(Wk[perm, :].T),
        "wv8": f8c(Wv[perm, :].T),
        "wmT": bf(Wm[:, perm].T),
        "w1xT": bf(W1.T[0:D, :]),
        "w1mT": bf(W1.T[D:2 * D, :]),
        "w2T": bf(W2.T),
        "bq": f32(bq[perm].reshape(2, 128).T),
        "bvb": bvb,
        "b1hl": np.ascontiguousarray(np.stack([b1hi, b1lo])),
        "b2hl": np.ascontiguousarray(np.stack([b2hi, b2lo])),
    }
    in_maps = []
    for core in range(NCORES):
        b, s = core // 4, core % 4
        m = dict(shared)
        xs = x[b][:, s * NS:(s + 1) * NS]
        m["xs8"] = f8c(xs)
        m["xsb"] = bf(xs)
        m["src8"] = f8c(source[b])
        in_maps.append(m)
    return in_maps


def run(inputs, **spmd_kwargs):
    """Build (cached), run on cores 0-7, return (full_output, BassKernelResults)."""
    nc = _build()
    in_maps = _prep_inputs(**inputs)
    res = bass_utils.run_bass_kernel_spmd(
        nc, in_maps, core_ids=list(range(NCORES)), **spmd_kwargs)
    full = np.empty((B, D, N), dtype=np.float32)
    for core in range(NCORES):
        b, s = core // 4, core % 4
        full[b][:, s * NS:(s + 1) * NS] = res.results[core]["out"]
    return full, res


def kernel(**inputs):
    full, _ = run(inputs)
    return full


# revision 30
# speedup vs baseline: 1.5410x; 1.0858x over previous
# Trainium2 Bass kernel for nn_AttentionalPropagation (B=2, D=256, N=M=4096, H=4).
#
# Sharding: 8 cores; each batch (B=2) owns 4 cores; each core computes a
# 1024-column sequence shard of the output end-to-end. k,v are computed
# redundantly per core from the full `source` of its batch. Cross-core
# communication: one AllGather of InstanceNorm partial (sum, sumsq) stats
# within each 4-core batch group (+ local sum), cheaper in the cost model
# than AllReduce.
#
# Engine plan (per core, cost-model driven):
#  - PE: all projections as fp8 DoubleRow (host-side fp8 conversion of
#    x/source/Wq/Wk/Wv), fp8 DoubleRow scores, mixed fp8-DoubleRow/bf16
#    attention, bf16 msg/h1/out. Biases enter via DVE evictions (q, v),
#    hi/lo bf16 ones-row matmul passes (b1', b2), or cancel entirely
#    (bk shifts each softmax column by a constant over m -> dropped;
#    bm is folded into b1' host-side).
#  - Softmax: scores/8 - 1 (offset keeps fp8 exp in range; cancels in the
#    normalization). exp is split between ACT (fp8 output, feeds DoubleRow
#    attn passes) and DVE (Schraudolph int16 bit-trick -> bf16 probs, feeds
#    plain bf16 attn passes). Denominator via the ones-column in v^T.
#  - Evictions that need no arithmetic go over DMA queues (msg, h1, out).

import os

import numpy as np

import concourse.bass as bass  # noqa: F401
import concourse.tile as tile
import concourse.mybir as mybir
from concourse import bacc
from concourse import bass_utils

B, D, N = 2, 256, 4096
H, DH = 4, 64
NS = N // 4           # sequence shard per core
NCORES = 8
EPS = 1e-5

FP = mybir.dt.float32
BF = mybir.dt.bfloat16
F8 = mybir.dt.float8e4
I16 = mybir.dt.int16
OP = mybir.AluOpType
AF = mybir.ActivationFunctionType
DR = mybir.MatmulPerfMode.DoubleRow

JA = 20               # exp chunks per group on ACT (fp8, DoubleRow attn)
JD = 32 - JA          # exp chunks per group on DVE (Schraudolph bf16)
# Interleave ACT/DVE ownership over the 16 m-chunk PAIRS of a group so both
# engines run concurrently (ACT pairs feed DoubleRow attn; DVE pairs bf16).
_NPAIR, _NDP = 16, JD // 2
_DVE_PAIRS = sorted({int(round((i + 0.5) * _NPAIR / _NDP - 0.5)) for i in range(_NDP)})
assert len(_DVE_PAIRS) == _NDP
C0 = 1.0              # exp offset: probs = exp(s/8 - C0); cancels in softmax
LOG2E = 1.4426950408889634
SCH_A = 0.125 * 128 * LOG2E                       # i16 = s*SCH_A + SCH_B
SCH_B = 127.0 * 128 - 128 * C0 * LOG2E - 0.5      # -0.5 centers truncation

_STAGE = os.environ.get("KSTAGE", "full")  # debug bisection: qk|attn|h1|full
_ALLGATHER = os.environ.get("KAG", "1") == "1"  # stats exchange: AllGather vs AllReduce


def _emit(nc, tc, io, es):
    out = io["out"]

    wpool = es.enter_context(tc.tile_pool(name="weights", bufs=1))
    apool = es.enter_context(tc.tile_pool(name="acts", bufs=1))

    # ---------- weight / bias / input loads ----------
    wq_sb = wpool.tile([128, 2, D], F8)
    nc.sync.dma_start(out=wq_sb[:], in_=io["wq8"].rearrange("(c p) o -> p c o", p=128))
    wk_sb = wpool.tile([128, 2, D], F8)
    nc.sync.dma_start(out=wk_sb[:], in_=io["wk8"].rearrange("(c p) o -> p c o", p=128))
    wv_sb = wpool.tile([128, 2, D], F8)
    nc.sync.dma_start(out=wv_sb[:], in_=io["wv8"].rearrange("(c p) o -> p c o", p=128))
    wm_sb = wpool.tile([128, 2, D], BF)
    nc.gpsimd.dma_start(out=wm_sb[:], in_=io["wmT"].rearrange("(c p) o -> p c o", p=128))
    w1x_sb = wpool.tile([128, 2, 2 * D], BF)
    nc.sync.dma_start(out=w1x_sb[:], in_=io["w1xT"].rearrange("(c p) o -> p c o", p=128))
    w1m_sb = wpool.tile([128, 2, 2 * D], BF)
    nc.gpsimd.dma_start(out=w1m_sb[:], in_=io["w1mT"].rearrange("(c p) o -> p c o", p=128))
    w2_sb = wpool.tile([128, 4, D], BF)
    nc.gpsimd.dma_start(out=w2_sb[:], in_=io["w2T"].rearrange("(c p) o -> p c o", p=128))

    bq_sb = wpool.tile([128, 2], FP)
    nc.sync.dma_start(out=bq_sb[:], in_=io["bq"][:])
    bvb_sb = wpool.tile([128, 4, 2, DH], FP)
    nc.sync.dma_start(
        out=bvb_sb[:], in_=io["bvb"].rearrange("p (h r d) -> p h r d", h=4, r=2))
    b1hl_sb = wpool.tile([2, 2 * D], BF)
    nc.sync.dma_start(out=b1hl_sb[:], in_=io["b1hl"][:])
    b2hl_sb = wpool.tile([2, D], BF)
    nc.sync.dma_start(out=b2hl_sb[:], in_=io["b2hl"][:])

    xs8_sb = apool.tile([128, 2, NS], F8)
    nc.sync.dma_start(out=xs8_sb[:], in_=io["xs8"].rearrange("(c p) n -> p c n", p=128))
    xsb_sb = apool.tile([128, 2, NS], BF)
    nc.sync.dma_start(out=xsb_sb[:], in_=io["xsb"].rearrange("(c p) n -> p c n", p=128))
    src8_sb = apool.tile([128, 2, N], F8)
    nc.sync.dma_start(out=src8_sb[:], in_=io["src8"].rearrange("(c p) m -> p c m", p=128))

    ones2 = wpool.tile([2, 512], BF)
    nc.vector.memset(ones2[:], 1.0)
    negc0 = wpool.tile([128, 1], FP)
    nc.vector.memset(negc0[:], -C0)

    # ---------- persistent activation tiles ----------
    qf_sb = apool.tile([128, 2, NS], F8)      # part = 64*hh + d, [kc, n]
    kf_sb = apool.tile([128, 2, N], F8)
    # DoubleRow score layout: one 32-partition tile per (kc, hh), dims [p, r, n]
    q8_t = [[apool.tile([32, 2, NS], F8, name=f"q8_{kc}{hh}") for hh in range(2)]
            for kc in range(2)]
    k8_t = [[apool.tile([32, 2, N], F8, name=f"k8_{kc}{hh}") for hh in range(2)]
            for kc in range(2)]
    # v^T per head + ones col, fp8, stride 80 for DoubleRow
    vaT_sb = apool.tile([128, H, 16, 2, 80], F8)
    exp8_sb = apool.tile([128, 2, JA // 2, 2, 512], F8)   # [., hh, p, r, n]
    prob16_sb = apool.tile([128, 2, JD, 512], I16)        # [., hh, jd, n]
    attn_sb = apool.tile([128, 2, NS], BF)
    msg_sb = apool.tile([128, 2, NS], BF)
    h1_sb = apool.tile([128, 4, NS], FP)
    h1n_sb = apool.tile([128, 4, NS], BF)
    bst_sb = apool.tile([128, 4, 2, 6], FP)
    stats_sb = apool.tile([128, 8], FP)

    nc.vector.memset(vaT_sb[:, :, :, :, DH:DH + 1], 1.0)

    # ---------- phase 1: projections (all fp8 DoubleRow) ----------
    with tc.tile_pool(name="pj", bufs=2, space="PSUM") as pj, \
         tc.tile_pool(name="vt", bufs=2, space="PSUM") as vtp:
        # q: per kc one DoubleRow matmul (contract 256 = 128 part x 2 ic)
        # q/k kc-major with reshuffle right after each kc so scores for the
        # first group (kc=0) can start while kc=1 and v still project.
        for kc in range(2):
            q_ps = pj.tile([128, NS], FP, tag="pj")
            for nh in range(2):
                nc.tensor.matmul(
                    q_ps[:, nh * 512:(nh + 1) * 512],
                    wq_sb[:, :, kc * 128:(kc + 1) * 128],
                    xs8_sb[:, :, nh * 512:(nh + 1) * 512],
                    start=True, stop=True, perf_mode=DR)
            nc.vector.tensor_scalar(
                out=qf_sb[:, kc, :], in0=q_ps[:],
                scalar1=bq_sb[:, kc:kc + 1], scalar2=None, op0=OP.add)
            for mq in range(4):
                k_ps = pj.tile([128, NS], FP, tag="pj")
                for mh in range(2):
                    m0 = mq * NS + mh * 512
                    nc.tensor.matmul(
                        k_ps[:, mh * 512:(mh + 1) * 512],
                        wk_sb[:, :, kc * 128:(kc + 1) * 128],
                        src8_sb[:, :, m0:m0 + 512],
                        start=True, stop=True, perf_mode=DR)
                nc.scalar.copy(kf_sb[:, kc, mq * NS:(mq + 1) * NS], k_ps[:])
            for hh in range(2):
                pi = 64 * hh
                for r in range(2):
                    nc.gpsimd.dma_start(
                        out=q8_t[kc][hh][:, r, :],
                        in_=qf_sb[pi + 32 * r:pi + 32 * r + 32, kc, :])
                    nc.gpsimd.dma_start(
                        out=k8_t[kc][hh][:, r, :],
                        in_=kf_sb[pi + 32 * r:pi + 32 * r + 32, kc, :])
        # v^T: per m-chunk of 128; out[m, (h d)]
        for g in range(8):
            vt_ps = vtp.tile([128, 2, 2, 4, DH], FP, tag="vt")
            for mm in range(4):
                mc = 4 * g + mm
                nc.tensor.matmul(
                    vt_ps[:, mm // 2, mm % 2, :, :],
                    src8_sb[:, :, mc * 128:(mc + 1) * 128], wv_sb[:],
                    start=True, stop=True, perf_mode=DR)
            for pb in range(2):
                nc.vector.tensor_tensor(
                    out=vaT_sb[:, :, 2 * g + pb, :, 0:DH],
                    in0=vt_ps[:, pb, :, :, :].transpose([0, 2, 1, 3]),
                    in1=bvb_sb[:], op=OP.add)

    if _STAGE == "qk":
        o_dbg = apool.tile([128, 2, NS], FP)
        nc.vector.tensor_copy(o_dbg[:, 0, :], qf_sb[:, 0, :])
        nc.vector.tensor_copy(o_dbg[:, 1, :], qf_sb[:, 1, :])
        nc.sync.dma_start(out=out.rearrange("(c p) n -> p c n", p=128), in_=o_dbg[:])
        return

    # ---------- phase 2+3: attention + chunkwise MLP pipeline ----------
    with tc.tile_pool(name="sc", bufs=2, space="PSUM") as scp, \
         tc.tile_pool(name="at", bufs=1, space="PSUM") as atp, \
         tc.tile_pool(name="mm", bufs=2, space="PSUM") as mmp, \
         tc.tile_pool(name="nrm", bufs=2) as nrm:

        def mlp_items(nch):
            """Deferred-emission MLP work items for column chunk `nch`,
            injected into the NEXT group's score stream so PE never idles."""
            n0 = nch * 512

            def msg_item(oc):
                def emit():
                    m_ps = mmp.tile([128, 512], FP, tag="mm")
                    for ic in range(2):
                        nc.tensor.matmul(
                            m_ps[:], wm_sb[:, ic, oc * 128:(oc + 1) * 128],
                            attn_sb[:, ic, n0:n0 + 512],
                            start=(ic == 0), stop=(ic == 1))
                    nc.vector.tensor_copy(msg_sb[:, oc, n0:n0 + 512], m_ps[:])
                return emit

            def h1_item(t):
                def emit():
                    h_ps = mmp.tile([128, 512], FP, tag="mm")
                    for ic in range(2):
                        nc.tensor.matmul(
                            h_ps[:], w1x_sb[:, ic, t * 128:(t + 1) * 128],
                            xsb_sb[:, ic, n0:n0 + 512], start=(ic == 0), stop=False)
                    for ic in range(2):
                        nc.tensor.matmul(
                            h_ps[:], w1m_sb[:, ic, t * 128:(t + 1) * 128],
                            msg_sb[:, ic, n0:n0 + 512], start=False, stop=False)
                    nc.tensor.matmul(
                        h_ps[:], b1hl_sb[:, t * 128:(t + 1) * 128], ones2[:],
                        start=False, stop=True)
                    nc.vector.tensor_copy(h1_sb[:, t, n0:n0 + 512], h_ps[:])
                    nc.vector.bn_stats(
                        out=bst_sb[:, t, nch, :], in_=h1_sb[:, t, n0:n0 + 512])
                return emit

            return [msg_item(0), msg_item(1)] + [h1_item(t) for t in range(4)]

        pending_mlp = []
        for nch in range(2):
            n0 = nch * 512
            for kc in range(2):
                at0 = atp.tile([128, 512], FP, tag="at0")
                at1 = atp.tile([128, 512], FP, tag="at1")
                ats = (at0, at1)
                sched = []  # (engine, pair t, ordinal)
                na = nd = 0
                for t in range(_NPAIR):
                    if t in _DVE_PAIRS:
                        sched.append(("D", t, nd)); nd += 1
                    else:
                        sched.append(("A", t, na)); na += 1
                # chunk-level emission order: merge the two j-streams so ACT
                # stays saturated while DVE consumes concurrently (2 PSUM bufs)
                a_js = [(2 * t + r, o, r) for e, t, o in sched if e == "A" for r in range(2)]
                d_js = [(2 * t + r, o, r) for e, t, o in sched if e == "D" for r in range(2)]
                order = []
                ca = cd = 0
                for _ in range(2 * _NPAIR):
                    if cd * 2 * JA <= ca * 2 * JD and cd < len(d_js):
                        order.append(("D",) + d_js[cd]); cd += 1
                    elif ca < len(a_js):
                        order.append(("A",) + a_js[ca]); ca += 1
                    else:
                        order.append(("D",) + d_js[cd]); cd += 1

                # attention passes become ready as their prob chunks complete;
                # emit them (and deferred MLP items) interleaved into the
                # score stream, trailing by 2 chunks.
                npass = [0, 0]
                nready = []            # queued pass emitters

                def attn_pass(eng, j, o, r):
                    def emit_for(hh):
                        h = 2 * kc + hh
                        at = ats[hh]
                        i = npass[hh]
                        npass[hh] += 1
                        last = (i == (_NPAIR - _NDP) + JD - 1)
                        if eng == "A":
                            nc.tensor.matmul(
                                at[:DH + 1, :], vaT_sb[:, h, j // 2, :, 0:DH + 1],
                                exp8_sb[:, hh, o, :, :],
                                start=(i == 0), stop=last, perf_mode=DR)
                        else:
                            nc.tensor.matmul(
                                at[:DH + 1, :], vaT_sb[:, h, j // 2, j % 2, 0:DH + 1],
                                prob16_sb[:, hh, 2 * o + r, :].bitcast(BF),
                                start=(i == 0), stop=last)
                    return emit_for

                mlp_slots = set(range(3, 3 + 5 * len(pending_mlp), 5))
                for pos, (eng, j, o, r) in enumerate(order):
                    sc_ps = scp.tile([128, 2, 512], FP, tag="sc")
                    for hh in range(2):
                        nc.tensor.matmul(
                            sc_ps[:, hh, :],
                            k8_t[kc][hh][:, :, j * 128:(j + 1) * 128],
                            q8_t[kc][hh][:, :, n0:n0 + 512],
                            start=True, stop=True, perf_mode=DR)
                    if eng == "A":
                        nc.scalar.activation(
                            out=exp8_sb[:, :, o, r, :], in_=sc_ps[:],
                            func=AF.Exp, scale=0.125, bias=negc0[:])
                        if r == 1:
                            nready.append(attn_pass(eng, j, o, r))
                    else:
                        nc.vector.tensor_scalar(
                            out=prob16_sb[:, :, 2 * o + r, :], in0=sc_ps[:],
                            scalar1=SCH_A, scalar2=SCH_B, op0=OP.mult, op1=OP.add)
                        nready.append(attn_pass(eng, j, o, r))
                    if pos in mlp_slots and pending_mlp:
                        pending_mlp.pop(0)()
                    # trail the pass emission ~3 chunks behind availability
                    while len(nready) > 3:
                        p = nready.pop(0)
                        p(0); p(1)
                for p in nready:
                    p(0); p(1)
                for hh in range(2):
                    at = ats[hh]
                    rz = nrm.tile([1, 512], FP, tag="rz")
                    nc.vector.reciprocal(rz[:], at[DH:DH + 1, :])
                    rzb = nrm.tile([DH, 512], FP, tag="rzb")
                    nc.gpsimd.partition_broadcast(rzb[:], rz[:])
                    nc.vector.tensor_mul(
                        attn_sb[64 * hh:64 * hh + DH, kc, n0:n0 + 512],
                        at[0:DH, :], rzb[:])
            pending_mlp.extend(mlp_items(nch))
        for item in pending_mlp:
            item()

        if _STAGE == "attn":
            o_dbg = apool.tile([128, 2, NS], FP)
            nc.vector.tensor_copy(o_dbg[:, 0, :], attn_sb[:, 0, :])
            nc.vector.tensor_copy(o_dbg[:, 1, :], attn_sb[:, 1, :])
            nc.sync.dma_start(out=out.rearrange("(c p) n -> p c n", p=128), in_=o_dbg[:])
            return
        if _STAGE == "h1":
            o_dbg = apool.tile([128, 2, NS], FP)
            nc.vector.tensor_copy(o_dbg[:, 0, :], h1_sb[:, 0, :])
            nc.vector.tensor_copy(o_dbg[:, 1, :], h1_sb[:, 1, :])
            nc.sync.dma_start(out=out.rearrange("(c p) n -> p c n", p=128), in_=o_dbg[:])
            return

        # ---------- instance-norm stats exchange + output ----------
        with tc.tile_pool(name="dram", bufs=1, space="DRAM") as dram, \
             tc.tile_pool(name="nstat", bufs=1) as nstat:
            # local (sum, sumsq) per channel from bn stats
            for t in range(4):
                mv = nstat.tile([128, 2], FP, tag="mv")
                nc.vector.bn_aggr(out=mv[:], in_=bst_sb[:, t, :, :])
                nc.vector.tensor_scalar_mul(stats_sb[:, t:t + 1], mv[:, 0:1], float(NS))
                msq = nstat.tile([128, 1], FP, tag="msq")
                nc.vector.tensor_mul(msq[:], mv[:, 0:1], mv[:, 0:1])
                msq2 = nstat.tile([128, 1], FP, tag="msq2")
                nc.vector.tensor_add(msq2[:], mv[:, 1:2], msq[:])
                nc.vector.tensor_scalar_mul(stats_sb[:, 4 + t:5 + t], msq2[:], float(NS))

            cc_in = dram.tile([128, 8], FP)
            if _ALLGATHER:
                cc_out = dram.tile([4, 128, 8], FP)
                nc.sync.dma_start(out=cc_in[:], in_=stats_sb[:])
                nc.gpsimd.collective_compute(
                    "AllGather", OP.bypass,
                    replica_groups=[[0, 1, 2, 3], [4, 5, 6, 7]],
                    ins=[cc_in[:].opt()], outs=[cc_out[:].opt()],
                )
                sred4 = nstat.tile([128, 4, 8], FP)
                nc.sync.dma_start(out=sred4[:], in_=cc_out.rearrange("k p s -> p k s"))
                s01 = nstat.tile([128, 8], FP)
                nc.vector.tensor_add(s01[:], sred4[:, 0, :], sred4[:, 1, :])
                s23 = nstat.tile([128, 8], FP)
                nc.vector.tensor_add(s23[:], sred4[:, 2, :], sred4[:, 3, :])
                sred = nstat.tile([128, 8], FP)
                nc.vector.tensor_add(sred[:], s01[:], s23[:])
            else:
                cc_out = dram.tile([128, 8], FP)
                nc.sync.dma_start(out=cc_in[:], in_=stats_sb[:])
                nc.gpsimd.collective_compute(
                    "AllReduce", OP.add,
                    replica_groups=[[0, 1, 2, 3], [4, 5, 6, 7]],
                    ins=[cc_in[:].opt()], outs=[cc_out[:].opt()],
                )
                sred = nstat.tile([128, 8], FP)
                nc.sync.dma_start(out=sred[:], in_=cc_out[:])

            mu4 = nstat.tile([128, 4], FP)
            nc.vector.tensor_scalar_mul(mu4[:], sred[:, 0:4], 1.0 / N)
            e24 = nstat.tile([128, 4], FP)
            nc.vector.tensor_scalar_mul(e24[:], sred[:, 4:8], 1.0 / N)
            var4 = nstat.tile([128, 4], FP)
            nc.vector.tensor_mul(var4[:], mu4[:], mu4[:])
            nc.vector.tensor_tensor(out=var4[:], in0=e24[:], in1=var4[:], op=OP.subtract)
            eps1 = nstat.tile([128, 1], FP)
            nc.vector.memset(eps1[:], EPS)
            std4 = nstat.tile([128, 4], FP)
            nc.scalar.activation(out=std4[:], in_=var4[:], func=AF.Sqrt, bias=eps1[:])
            rstd4 = nstat.tile([128, 4], FP)
            nc.vector.reciprocal(rstd4[:], std4[:])
            nb4 = nstat.tile([128, 4], FP)
            nc.vector.tensor_mul(nb4[:], mu4[:], rstd4[:])
            nc.vector.tensor_scalar_mul(nb4[:], nb4[:], -1.0)

            # h = relu(h1 * rstd - mu * rstd)
            for t in range(4):
                nc.scalar.activation(
                    out=h1n_sb[:, t, :], in_=h1_sb[:, t, :], func=AF.Relu,
                    bias=nb4[:, t:t + 1], scale=rstd4[:, t:t + 1])

            # out = W2T.T @ h + b2 (ones-row hi/lo pass)
            out_sb = apool.tile([128, 2, NS], FP)
            outp = out.rearrange("(c p) n -> p c n", p=128)
            for oc in range(2):
                for nch in range(2):
                    n0 = nch * 512
                    o_ps = mmp.tile([128, 512], FP, tag="mm")
                    for kc2 in range(4):
                        nc.tensor.matmul(
                            o_ps[:], w2_sb[:, kc2, oc * 128:(oc + 1) * 128],
                            h1n_sb[:, kc2, n0:n0 + 512],
                            start=(kc2 == 0), stop=False)
                    nc.tensor.matmul(
                        o_ps[:], b2hl_sb[:, oc * 128:(oc + 1) * 128], ones2[:],
                        start=False, stop=True)
                    nc.vector.tensor_copy(out_sb[:, oc, n0:n0 + 512], o_ps[:])
                    nc.sync.dma_start(out=outp[:, oc, n0:n0 + 512], in_=out_sb[:, oc, n0:n0 + 512])


_BUILT = {}


def _build():
    if "nc" in _BUILT:
        return _BUILT["nc"]
    nc = bacc.Bacc("TRN2", target_bir_lowering=False, debug=False,
                   enable_asserts=True, num_devices=NCORES)
    io = {}
    io["xs8"] = nc.dram_tensor("xs8", [D, NS], F8, kind="ExternalInput").ap()
    io["xsb"] = nc.dram_tensor("xsb", [D, NS], BF, kind="ExternalInput").ap()
    io["src8"] = nc.dram_tensor("src8", [D, N], F8, kind="ExternalInput").ap()
    io["wq8"] = nc.dram_tensor("wq8", [D, D], F8, kind="ExternalInput").ap()
    io["wk8"] = nc.dram_tensor("wk8", [D, D], F8, kind="ExternalInput").ap()
    io["wv8"] = nc.dram_tensor("wv8", [D, D], F8, kind="ExternalInput").ap()
    io["wmT"] = nc.dram_tensor("wmT", [D, D], BF, kind="ExternalInput").ap()
    io["w1xT"] = nc.dram_tensor("w1xT", [D, 2 * D], BF, kind="ExternalInput").ap()
    io["w1mT"] = nc.dram_tensor("w1mT", [D, 2 * D], BF, kind="ExternalInput").ap()
    io["w2T"] = nc.dram_tensor("w2T", [2 * D, D], BF, kind="ExternalInput").ap()
    io["bq"] = nc.dram_tensor("bq", [128, 2], FP, kind="ExternalInput").ap()
    io["bvb"] = nc.dram_tensor("bvb", [128, 8 * DH], FP, kind="ExternalInput").ap()
    io["b1hl"] = nc.dram_tensor("b1hl", [2, 2 * D], BF, kind="ExternalInput").ap()
    io["b2hl"] = nc.dram_tensor("b2hl", [2, D], BF, kind="ExternalInput").ap()
    io["out"] = nc.dram_tensor("out", [D, NS], FP, kind="ExternalOutput").ap()

    import contextlib
    with tile.TileContext(nc) as tc:
        with contextlib.ExitStack() as es:
            _emit(nc, tc, io, es)
    nc.compile()
    _BUILT["nc"] = nc
    return nc


def _prep_inputs(x, source, Wq, bq, Wk, bk, Wv, bv, Wm, bm, W1, b1, W2, b2):
    import ml_dtypes
    npF8 = mybir.dt.np(F8)
    npBF = ml_dtypes.bfloat16
    perm = np.array([4 * d + h for h in range(H) for d in range(DH)])
    f32 = lambda a: np.ascontiguousarray(a, dtype=np.float32)
    bf = lambda a: np.ascontiguousarray(np.asarray(a, np.float32), dtype=npBF)
    f8c = lambda a: np.ascontiguousarray(np.asarray(a, np.float32), dtype=npF8)

    b1p = np.asarray(b1, np.float64) + np.asarray(W1, np.float64)[:, D:] @ np.asarray(bm, np.float64)
    b1hi = np.asarray(b1p, np.float32).astype(npBF)
    b1lo = (np.asarray(b1p, np.float32) - b1hi.astype(np.float32)).astype(npBF)
    b2hi = np.asarray(b2, np.float32).astype(npBF)
    b2lo = (np.asarray(b2, np.float32) - b2hi.astype(np.float32)).astype(npBF)

    bvp = np.asarray(bv, np.float32)[perm]                    # (h, d) order
    bvb = np.tile(bvp.reshape(4, 1, DH), (1, 2, 1)).reshape(1, 8 * DH)
    bvb = np.ascontiguousarray(np.broadcast_to(bvb, (128, 8 * DH)), np.float32)

    shared = {
        "wq8": f8c(Wq[perm, :].T),
        "wk8": f8c(Wk[perm, :].T),
        "wv8": f8c(Wv[perm, :].T),
        "wmT": bf(Wm[:, perm].T),
        "w1xT": bf(W1.T[0:D, :]),
        "w1mT": bf(W1.T[D:2 * D, :]),
        "w2T": bf(W2.T),
        "bq": f32(bq[perm].reshape(2, 128).T),
        "bvb": bvb,
        "b1hl": np.ascontiguousarray(np.stack([b1hi, b1lo])),
        "b2hl": np.ascontiguousarray(np.stack([b2hi, b2lo])),
    }
    in_maps = []
    for core in range(NCORES):
        b, s = core // 4, core % 4
        m = dict(shared)
        xs = x[b][:, s * NS:(s + 1) * NS]
        m["xs8"] = f8c(xs)
        m["xsb"] = bf(xs)
        m["src8"] = f8c(source[b])
        in_maps.append(m)
    return in_maps


def run(inputs, **spmd_kwargs):
    """Build (cached), run on cores 0-7, return (full_output, BassKernelResults)."""
    nc = _build()
    in_maps = _prep_inputs(**inputs)
    res = bass_utils.run_bass_kernel_spmd(
        nc, in_maps, core_ids=list(range(NCORES)), **spmd_kwargs)
    full = np.empty((B, D, N), dtype=np.float32)
    for core in range(NCORES):
        b, s = core // 4, core % 4
        full[b][:, s * NS:(s + 1) * NS] = res.results[core]["out"]
    return full, res


def kernel(**inputs):
    full, _ = run(inputs)
    return full


# revision 34
# speedup vs baseline: 1.5599x; 1.0123x over previous
# Trainium2 Bass kernel for nn_AttentionalPropagation (B=2, D=256, N=M=4096, H=4).
#
# Sharding: 8 cores; each batch (B=2) owns 4 cores; each core computes a
# 1024-column sequence shard of the output end-to-end. k,v are computed
# redundantly per core from the full `source` of its batch. Cross-core
# communication: one AllGather of InstanceNorm partial (sum, sumsq) stats
# within each 4-core batch group (+ local sum), cheaper in the cost model
# than AllReduce.
#
# Engine plan (per core, cost-model driven):
#  - PE: all projections as fp8 DoubleRow (host-side fp8 conversion of
#    x/source/Wq/Wk/Wv), fp8 DoubleRow scores, mixed fp8-DoubleRow/bf16
#    attention, bf16 msg/h1/out. Biases enter via DVE evictions (q, v),
#    hi/lo bf16 ones-row matmul passes (b1', b2), or cancel entirely
#    (bk shifts each softmax column by a constant over m -> dropped;
#    bm is folded into b1' host-side).
#  - Softmax: scores/8 - 1 (offset keeps fp8 exp in range; cancels in the
#    normalization). exp is split between ACT (fp8 output, feeds DoubleRow
#    attn passes) and DVE (Schraudolph int16 bit-trick -> bf16 probs, feeds
#    plain bf16 attn passes). Denominator via the ones-column in v^T.
#  - Evictions that need no arithmetic go over DMA queues (msg, h1, out).

import os

import numpy as np

import concourse.bass as bass  # noqa: F401
import concourse.tile as tile
import concourse.mybir as mybir
from concourse import bacc
from concourse import bass_utils

B, D, N = 2, 256, 4096
H, DH = 4, 64
NS = N // 4           # sequence shard per core
NCORES = 8
EPS = 1e-5

FP = mybir.dt.float32
BF = mybir.dt.bfloat16
F8 = mybir.dt.float8e4
I16 = mybir.dt.int16
OP = mybir.AluOpType
AF = mybir.ActivationFunctionType
DR = mybir.MatmulPerfMode.DoubleRow

JA = 20               # exp chunks per group on ACT (fp8, DoubleRow attn)
JD = 32 - JA          # exp chunks per group on DVE (Schraudolph bf16)
# Interleave ACT/DVE ownership over the 16 m-chunk PAIRS of a group so both
# engines run concurrently (ACT pairs feed DoubleRow attn; DVE pairs bf16).
_NPAIR, _NDP = 16, JD // 2
_DVE_PAIRS = sorted({int(round((i + 0.5) * _NPAIR / _NDP - 0.5)) for i in range(_NDP)})
assert len(_DVE_PAIRS) == _NDP
C0 = 1.0              # exp offset: probs = exp(s/8 - C0); cancels in softmax
LOG2E = 1.4426950408889634
SCH_A = 0.125 * 128 * LOG2E                       # i16 = s*SCH_A + SCH_B
SCH_B = 127.0 * 128 - 128 * C0 * LOG2E - 0.5      # -0.5 centers truncation

_STAGE = os.environ.get("KSTAGE", "full")  # debug bisection: qk|attn|h1|full
_ALLGATHER = os.environ.get("KAG", "1") == "1"  # stats exchange: AllGather vs AllReduce


def _emit(nc, tc, io, es):
    out = io["out"]

    wpool = es.enter_context(tc.tile_pool(name="weights", bufs=1))
    apool = es.enter_context(tc.tile_pool(name="acts", bufs=1))

    # ---------- weight / bias / input loads ----------
    # critical path first (q/k/v projections): xs8, src8, wq, wk, wv on the
    # sync queue; everything else on the gpsimd queue.
    xs8_sb = apool.tile([128, 2, NS], F8)
    nc.sync.dma_start(out=xs8_sb[:], in_=io["xs8"].rearrange("(c p) n -> p c n", p=128))
    wq_sb = wpool.tile([128, 2, D], F8)
    nc.sync.dma_start(out=wq_sb[:], in_=io["wq8"].rearrange("(c p) o -> p c o", p=128))
    wk_sb = wpool.tile([128, 2, D], F8)
    nc.sync.dma_start(out=wk_sb[:], in_=io["wk8"].rearrange("(c p) o -> p c o", p=128))
    src8_sb = apool.tile([128, 2, N], F8)
    nc.sync.dma_start(out=src8_sb[:], in_=io["src8"].rearrange("(c p) m -> p c m", p=128))
    wv_sb = wpool.tile([128, 2, D], F8)
    nc.sync.dma_start(out=wv_sb[:], in_=io["wv8"].rearrange("(c p) o -> p c o", p=128))
    bq_sb = wpool.tile([128, 2], FP)
    nc.sync.dma_start(out=bq_sb[:], in_=io["bq"][:])
    bvb_sb = wpool.tile([128, 4, 2, DH], FP)
    nc.sync.dma_start(
        out=bvb_sb[:], in_=io["bvb"].rearrange("p (h r d) -> p h r d", h=4, r=2))

    wm_sb = wpool.tile([128, 2, D], BF)
    nc.gpsimd.dma_start(out=wm_sb[:], in_=io["wmT"].rearrange("(c p) o -> p c o", p=128))
    w1x_sb = wpool.tile([128, 2, 2 * D], BF)
    nc.gpsimd.dma_start(out=w1x_sb[:], in_=io["w1xT"].rearrange("(c p) o -> p c o", p=128))
    w1m_sb = wpool.tile([128, 2, 2 * D], BF)
    nc.gpsimd.dma_start(out=w1m_sb[:], in_=io["w1mT"].rearrange("(c p) o -> p c o", p=128))
    w2_sb = wpool.tile([128, 4, D], BF)
    nc.gpsimd.dma_start(out=w2_sb[:], in_=io["w2T"].rearrange("(c p) o -> p c o", p=128))
    b1hl_sb = wpool.tile([2, 2 * D], BF)
    nc.gpsimd.dma_start(out=b1hl_sb[:], in_=io["b1hl"][:])
    b2hl_sb = wpool.tile([2, D], BF)
    nc.gpsimd.dma_start(out=b2hl_sb[:], in_=io["b2hl"][:])
    xsb_sb = apool.tile([128, 2, NS], BF)
    nc.gpsimd.dma_start(out=xsb_sb[:], in_=io["xsb"].rearrange("(c p) n -> p c n", p=128))

    ones2 = wpool.tile([2, 512], BF)
    nc.vector.memset(ones2[:], 1.0)
    negc0 = wpool.tile([128, 1], FP)
    nc.vector.memset(negc0[:], -C0)

    # ---------- persistent activation tiles ----------
    qf_sb = apool.tile([128, 2, NS], F8)      # part = 64*hh + d, [kc, n]
    kf_sb = apool.tile([128, 2, N], F8)
    # DoubleRow score layout: one 32-partition tile per (kc, hh), dims [p, r, n]
    q8_t = [[apool.tile([32, 2, NS], F8, name=f"q8_{kc}{hh}") for hh in range(2)]
            for kc in range(2)]
    k8_t = [[apool.tile([32, 2, N], F8, name=f"k8_{kc}{hh}") for hh in range(2)]
            for kc in range(2)]
    # v^T per head + ones col, fp8, stride 80 for DoubleRow
    vaT_sb = apool.tile([128, H, 16, 2, 80], F8)
    exp8_sb = apool.tile([128, 2, JA // 2, 2, 512], F8)   # [., hh, p, r, n]
    prob16_sb = apool.tile([128, 2, JD, 512], I16)        # [., hh, jd, n]
    attn_sb = apool.tile([128, 2, NS], BF)
    msg_sb = apool.tile([128, 2, NS], BF)
    h1_sb = apool.tile([128, 4, NS], FP)
    h1n_sb = apool.tile([128, 4, NS], BF)
    bst_sb = apool.tile([128, 4, 2, 6], FP)
    stats_sb = apool.tile([128, 8], FP)

    nc.vector.memset(vaT_sb[:, :, :, :, DH:DH + 1], 1.0)

    # ---------- phase 1: projections (all fp8 DoubleRow) ----------
    with tc.tile_pool(name="pj", bufs=2, space="PSUM") as pj, \
         tc.tile_pool(name="vt", bufs=2, space="PSUM") as vtp:
        # q: per kc one DoubleRow matmul (contract 256 = 128 part x 2 ic)
        # q/k kc-major with reshuffle right after each kc so scores for the
        # first group (kc=0) can start while kc=1 and v still project.
        for kc in range(2):
            q_ps = pj.tile([128, NS], FP, tag="pj")
            for nh in range(2):
                nc.tensor.matmul(
                    q_ps[:, nh * 512:(nh + 1) * 512],
                    wq_sb[:, :, kc * 128:(kc + 1) * 128],
                    xs8_sb[:, :, nh * 512:(nh + 1) * 512],
                    start=True, stop=True, perf_mode=DR)
            nc.vector.tensor_scalar(
                out=qf_sb[:, kc, :], in0=q_ps[:],
                scalar1=bq_sb[:, kc:kc + 1], scalar2=None, op0=OP.add)
            for mq in range(4):
                k_ps = pj.tile([128, NS], FP, tag="pj")
                for mh in range(2):
                    m0 = mq * NS + mh * 512
                    nc.tensor.matmul(
                        k_ps[:, mh * 512:(mh + 1) * 512],
                        wk_sb[:, :, kc * 128:(kc + 1) * 128],
                        src8_sb[:, :, m0:m0 + 512],
                        start=True, stop=True, perf_mode=DR)
                nc.scalar.copy(kf_sb[:, kc, mq * NS:(mq + 1) * NS], k_ps[:])
            for hh in range(2):
                pi = 64 * hh
                for r in range(2):
                    nc.gpsimd.dma_start(
                        out=q8_t[kc][hh][:, r, :],
                        in_=qf_sb[pi + 32 * r:pi + 32 * r + 32, kc, :])
                    nc.gpsimd.dma_start(
                        out=k8_t[kc][hh][:, r, :],
                        in_=kf_sb[pi + 32 * r:pi + 32 * r + 32, kc, :])
        # v^T: per m-chunk of 128; out[m, (h d)]
        for g in range(8):
            vt_ps = vtp.tile([128, 2, 2, 4, DH], FP, tag="vt")
            for mm in range(4):
                mc = 4 * g + mm
                nc.tensor.matmul(
                    vt_ps[:, mm // 2, mm % 2, :, :],
                    src8_sb[:, :, mc * 128:(mc + 1) * 128], wv_sb[:],
                    start=True, stop=True, perf_mode=DR)
            for pb in range(2):
                nc.vector.tensor_tensor(
                    out=vaT_sb[:, :, 2 * g + pb, :, 0:DH],
                    in0=vt_ps[:, pb, :, :, :].transpose([0, 2, 1, 3]),
                    in1=bvb_sb[:], op=OP.add)

    if _STAGE == "qk":
        o_dbg = apool.tile([128, 2, NS], FP)
        nc.vector.tensor_copy(o_dbg[:, 0, :], qf_sb[:, 0, :])
        nc.vector.tensor_copy(o_dbg[:, 1, :], qf_sb[:, 1, :])
        nc.sync.dma_start(out=out.rearrange("(c p) n -> p c n", p=128), in_=o_dbg[:])
        return

    # ---------- phase 2+3: attention + chunkwise MLP pipeline ----------
    with tc.tile_pool(name="sc", bufs=2, space="PSUM") as scp, \
         tc.tile_pool(name="at", bufs=1, space="PSUM") as atp, \
         tc.tile_pool(name="mm", bufs=2, space="PSUM") as mmp, \
         tc.tile_pool(name="nrm", bufs=2) as nrm, \
         tc.tile_pool(name="dram", bufs=1, space="DRAM") as dram, \
         tc.tile_pool(name="nstat", bufs=1) as nstat:

        cc_in = [dram.tile([128, 8], FP, name=f"cc_in{i}") for i in range(2)]
        cc_out = [dram.tile([4, 128, 8], FP, name=f"cc_out{i}") for i in range(2)]

        def emit_stats_cc(nch):
            """Per-nch (sum, sumsq) conversion + AllGather; nch=0 is issued
            mid-kernel so its collective hides under groups 2-3."""
            for t in range(4):
                mv = nstat.tile([128, 2], FP, tag="mv")
                nc.vector.bn_aggr(out=mv[:], in_=bst_sb[:, t, nch:nch + 1, :])
                nc.vector.tensor_scalar_mul(
                    stats_sb[:, t:t + 1], mv[:, 0:1], 512.0)
                msq = nstat.tile([128, 1], FP, tag="msq")
                nc.vector.tensor_mul(msq[:], mv[:, 0:1], mv[:, 0:1])
                msq2 = nstat.tile([128, 1], FP, tag="msq2")
                nc.vector.tensor_add(msq2[:], mv[:, 1:2], msq[:])
                nc.vector.tensor_scalar_mul(stats_sb[:, 4 + t:5 + t], msq2[:], 512.0)
            nc.sync.dma_start(out=cc_in[nch][:], in_=stats_sb[:])
            nc.gpsimd.collective_compute(
                "AllGather", OP.bypass,
                replica_groups=[[0, 1, 2, 3], [4, 5, 6, 7]],
                ins=[cc_in[nch][:].opt()], outs=[cc_out[nch][:].opt()],
            )

        def mlp_items(nch):
            """Deferred-emission MLP work items for column chunk `nch`,
            injected into the NEXT group's score stream so PE never idles."""
            n0 = nch * 512

            def msg_item(oc):
                def emit():
                    m_ps = mmp.tile([128, 512], FP, tag="mm")
                    for ic in range(2):
                        nc.tensor.matmul(
                            m_ps[:], wm_sb[:, ic, oc * 128:(oc + 1) * 128],
                            attn_sb[:, ic, n0:n0 + 512],
                            start=(ic == 0), stop=(ic == 1))
                    nc.vector.tensor_copy(msg_sb[:, oc, n0:n0 + 512], m_ps[:])
                return emit

            def h1_item(t):
                def emit():
                    h_ps = mmp.tile([128, 512], FP, tag="mm")
                    for ic in range(2):
                        nc.tensor.matmul(
                            h_ps[:], w1x_sb[:, ic, t * 128:(t + 1) * 128],
                            xsb_sb[:, ic, n0:n0 + 512], start=(ic == 0), stop=False)
                    for ic in range(2):
                        nc.tensor.matmul(
                            h_ps[:], w1m_sb[:, ic, t * 128:(t + 1) * 128],
                            msg_sb[:, ic, n0:n0 + 512], start=False, stop=False)
                    nc.tensor.matmul(
                        h_ps[:], b1hl_sb[:, t * 128:(t + 1) * 128], ones2[:],
                        start=False, stop=True)
                    nc.vector.tensor_copy(h1_sb[:, t, n0:n0 + 512], h_ps[:])
                    nc.vector.bn_stats(
                        out=bst_sb[:, t, nch, :], in_=h1_sb[:, t, n0:n0 + 512])
                return emit

            return [msg_item(0), msg_item(1)] + [h1_item(t) for t in range(4)]

        pending_mlp = []
        for nch in range(2):
            n0 = nch * 512
            for kc in range(2):
                at0 = atp.tile([128, 512], FP, tag="at0")
                at1 = atp.tile([128, 512], FP, tag="at1")
                ats = (at0, at1)
                sched = []  # (engine, pair t, ordinal)
                na = nd = 0
                for t in range(_NPAIR):
                    if t in _DVE_PAIRS:
                        sched.append(("D", t, nd)); nd += 1
                    else:
                        sched.append(("A", t, na)); na += 1
                # chunk-level emission order: merge the two j-streams so ACT
                # stays saturated while DVE consumes concurrently (2 PSUM bufs)
                a_js = [(2 * t + r, o, r) for e, t, o in sched if e == "A" for r in range(2)]
                d_js = [(2 * t + r, o, r) for e, t, o in sched if e == "D" for r in range(2)]
                order = []
                ca = cd = 0
                for _ in range(2 * _NPAIR):
                    if cd * 2 * JA <= ca * 2 * JD and cd < len(d_js):
                        order.append(("D",) + d_js[cd]); cd += 1
                    elif ca < len(a_js):
                        order.append(("A",) + a_js[ca]); ca += 1
                    else:
                        order.append(("D",) + d_js[cd]); cd += 1

                # attention passes become ready as their prob chunks complete;
                # emit them (and deferred MLP items) interleaved into the
                # score stream, trailing by 2 chunks.
                npass = [0, 0]
                nready = []            # queued pass emitters

                def attn_pass(eng, j, o, r):
                    def emit_for(hh):
                        h = 2 * kc + hh
                        at = ats[hh]
                        i = npass[hh]
                        npass[hh] += 1
                        last = (i == (_NPAIR - _NDP) + JD - 1)
                        if eng == "A":
                            nc.tensor.matmul(
                                at[:DH + 1, :], vaT_sb[:, h, j // 2, :, 0:DH + 1],
                                exp8_sb[:, hh, o, :, :],
                                start=(i == 0), stop=last, perf_mode=DR)
                        else:
                            nc.tensor.matmul(
                                at[:DH + 1, :], vaT_sb[:, h, j // 2, j % 2, 0:DH + 1],
                                prob16_sb[:, hh, 2 * o + r, :].bitcast(BF),
                                start=(i == 0), stop=last)
                    return emit_for

                mlp_slots = set(range(3, 3 + 5 * len(pending_mlp), 5))
                for pos, (eng, j, o, r) in enumerate(order):
                    sc_ps = scp.tile([128, 2, 512], FP, tag="sc")
                    for hh in range(2):
                        nc.tensor.matmul(
                            sc_ps[:, hh, :],
                            k8_t[kc][hh][:, :, j * 128:(j + 1) * 128],
                            q8_t[kc][hh][:, :, n0:n0 + 512],
                            start=True, stop=True, perf_mode=DR)
                    if eng == "A":
                        nc.scalar.activation(
                            out=exp8_sb[:, :, o, r, :], in_=sc_ps[:],
                            func=AF.Exp, scale=0.125, bias=negc0[:])
                        if r == 1:
                            nready.append(attn_pass(eng, j, o, r))
                    else:
                        nc.vector.tensor_scalar(
                            out=prob16_sb[:, :, 2 * o + r, :], in0=sc_ps[:],
                            scalar1=SCH_A, scalar2=SCH_B, op0=OP.mult, op1=OP.add)
                        nready.append(attn_pass(eng, j, o, r))
                    if pos in mlp_slots and pending_mlp:
                        pending_mlp.pop(0)()
                    # trail the pass emission ~3 chunks behind availability
                    while len(nready) > 3:
                        p = nready.pop(0)
                        p(0); p(1)
                for p in nready:
                    p(0); p(1)
                for hh in range(2):
                    at = ats[hh]
                    rz = nrm.tile([1, 512], FP, tag="rz")
                    nc.vector.reciprocal(rz[:], at[DH:DH + 1, :])
                    rzb = nrm.tile([DH, 512], FP, tag="rzb")
                    nc.gpsimd.partition_broadcast(rzb[:], rz[:])
                    nc.vector.tensor_mul(
                        attn_sb[64 * hh:64 * hh + DH, kc, n0:n0 + 512],
                        at[0:DH, :], rzb[:])
            pending_mlp.extend(mlp_items(nch))
            if nch == 0 and _ALLGATHER:
                pending_mlp.append(lambda: emit_stats_cc(0))
        for item in pending_mlp:
            item()

        if _STAGE == "attn":
            o_dbg = apool.tile([128, 2, NS], FP)
            nc.vector.tensor_copy(o_dbg[:, 0, :], attn_sb[:, 0, :])
            nc.vector.tensor_copy(o_dbg[:, 1, :], attn_sb[:, 1, :])
            nc.sync.dma_start(out=out.rearrange("(c p) n -> p c n", p=128), in_=o_dbg[:])
            return
        if _STAGE == "h1":
            o_dbg = apool.tile([128, 2, NS], FP)
            nc.vector.tensor_copy(o_dbg[:, 0, :], h1_sb[:, 0, :])
            nc.vector.tensor_copy(o_dbg[:, 1, :], h1_sb[:, 1, :])
            nc.sync.dma_start(out=out.rearrange("(c p) n -> p c n", p=128), in_=o_dbg[:])
            return

        # ---------- instance-norm stats exchange + output ----------
        if True:
            if _ALLGATHER:
                emit_stats_cc(1)
                sred8 = nstat.tile([128, 2, 4, 8], FP)
                for i in range(2):
                    nc.sync.dma_start(
                        out=sred8[:, i, :, :], in_=cc_out[i].rearrange("k p s -> p k s"))
                s02 = nstat.tile([128, 2, 8], FP)
                nc.vector.tensor_add(s02[:], sred8[:, :, 0, :], sred8[:, :, 1, :])
                s13 = nstat.tile([128, 2, 8], FP)
                nc.vector.tensor_add(s13[:], sred8[:, :, 2, :], sred8[:, :, 3, :])
                sredn = nstat.tile([128, 2, 8], FP)
                nc.vector.tensor_add(sredn[:], s02[:], s13[:])
                sred = nstat.tile([128, 8], FP)
                nc.vector.tensor_add(sred[:], sredn[:, 0, :], sredn[:, 1, :])
            else:
                # single AllReduce fallback over full-row stats
                for t in range(4):
                    mv = nstat.tile([128, 2], FP, tag="mv")
                    nc.vector.bn_aggr(out=mv[:], in_=bst_sb[:, t, :, :])
                    nc.vector.tensor_scalar_mul(stats_sb[:, t:t + 1], mv[:, 0:1], float(NS))
                    msq = nstat.tile([128, 1], FP, tag="msq")
                    nc.vector.tensor_mul(msq[:], mv[:, 0:1], mv[:, 0:1])
                    msq2 = nstat.tile([128, 1], FP, tag="msq2")
                    nc.vector.tensor_add(msq2[:], mv[:, 1:2], msq[:])
                    nc.vector.tensor_scalar_mul(stats_sb[:, 4 + t:5 + t], msq2[:], float(NS))
                cc_outr = dram.tile([128, 8], FP)
                nc.sync.dma_start(out=cc_in[0][:], in_=stats_sb[:])
                nc.gpsimd.collective_compute(
                    "AllReduce", OP.add,
                    replica_groups=[[0, 1, 2, 3], [4, 5, 6, 7]],
                    ins=[cc_in[0][:].opt()], outs=[cc_outr[:].opt()],
                )
                sred = nstat.tile([128, 8], FP)
                nc.sync.dma_start(out=sred[:], in_=cc_outr[:])

            mu4 = nstat.tile([128, 4], FP)
            nc.vector.tensor_scalar_mul(mu4[:], sred[:, 0:4], 1.0 / N)
            e24 = nstat.tile([128, 4], FP)
            nc.vector.tensor_scalar_mul(e24[:], sred[:, 4:8], 1.0 / N)
            var4 = nstat.tile([128, 4], FP)
            nc.vector.tensor_mul(var4[:], mu4[:], mu4[:])
            nc.vector.tensor_tensor(out=var4[:], in0=e24[:], in1=var4[:], op=OP.subtract)
            eps1 = nstat.tile([128, 1], FP)
            nc.vector.memset(eps1[:], EPS)
            std4 = nstat.tile([128, 4], FP)
            nc.scalar.activation(out=std4[:], in_=var4[:], func=AF.Sqrt, bias=eps1[:])
            rstd4 = nstat.tile([128, 4], FP)
            nc.vector.reciprocal(rstd4[:], std4[:])
            nb4 = nstat.tile([128, 4], FP)
            nc.vector.tensor_mul(nb4[:], mu4[:], rstd4[:])
            nc.vector.tensor_scalar_mul(nb4[:], nb4[:], -1.0)

            # h = relu(h1 * rstd - mu * rstd)
            for t in range(4):
                nc.scalar.activation(
                    out=h1n_sb[:, t, :], in_=h1_sb[:, t, :], func=AF.Relu,
                    bias=nb4[:, t:t + 1], scale=rstd4[:, t:t + 1])

            # out = W2T.T @ h + b2 (ones-row hi/lo pass)
            out_sb = apool.tile([128, 2, NS], FP)
            outp = out.rearrange("(c p) n -> p c n", p=128)
            for oc in range(2):
                for nch in range(2):
                    n0 = nch * 512
                    o_ps = mmp.tile([128, 512], FP, tag="mm")
                    for kc2 in range(4):
                        nc.tensor.matmul(
                            o_ps[:], w2_sb[:, kc2, oc * 128:(oc + 1) * 128],
                            h1n_sb[:, kc2, n0:n0 + 512],
                            start=(kc2 == 0), stop=False)
                    nc.tensor.matmul(
                        o_ps[:], b2hl_sb[:, oc * 128:(oc + 1) * 128], ones2[:],
                        start=False, stop=True)
                    nc.vector.tensor_copy(out_sb[:, oc, n0:n0 + 512], o_ps[:])
                    nc.sync.dma_start(out=outp[:, oc, n0:n0 + 512], in_=out_sb[:, oc, n0:n0 + 512])


_BUILT = {}


def _build():
    if "nc" in _BUILT:
        return _BUILT["nc"]
    nc = bacc.Bacc("TRN2", target_bir_lowering=False, debug=False,
                   enable_asserts=True, num_devices=NCORES)
    io = {}
    io["xs8"] = nc.dram_tensor("xs8", [D, NS], F8, kind="ExternalInput").ap()
    io["xsb"] = nc.dram_tensor("xsb", [D, NS], BF, kind="ExternalInput").ap()
    io["src8"] = nc.dram_tensor("src8", [D, N], F8, kind="ExternalInput").ap()
    io["wq8"] = nc.dram_tensor("wq8", [D, D], F8, kind="ExternalInput").ap()
    io["wk8"] = nc.dram_tensor("wk8", [D, D], F8, kind="ExternalInput").ap()
    io["wv8"] = nc.dram_tensor("wv8", [D, D], F8, kind="ExternalInput").ap()
    io["wmT"] = nc.dram_tensor("wmT", [D, D], BF, kind="ExternalInput").ap()
    io["w1xT"] = nc.dram_tensor("w1xT", [D, 2 * D], BF, kind="ExternalInput").ap()
    io["w1mT"] = nc.dram_tensor("w1mT", [D, 2 * D], BF, kind="ExternalInput").ap()
    io["w2T"] = nc.dram_tensor("w2T", [2 * D, D], BF, kind="ExternalInput").ap()
    io["bq"] = nc.dram_tensor("bq", [128, 2], FP, kind="ExternalInput").ap()
    io["bvb"] = nc.dram_tensor("bvb", [128, 8 * DH], FP, kind="ExternalInput").ap()
    io["b1hl"] = nc.dram_tensor("b1hl", [2, 2 * D], BF, kind="ExternalInput").ap()
    io["b2hl"] = nc.dram_tensor("b2hl", [2, D], BF, kind="ExternalInput").ap()
    io["out"] = nc.dram_tensor("out", [D, NS], FP, kind="ExternalOutput").ap()

    import contextlib
    with tile.TileContext(nc) as tc:
        with contextlib.ExitStack() as es:
            _emit(nc, tc, io, es)
    nc.compile()
    _BUILT["nc"] = nc
    return nc


def _prep_inputs(x, source, Wq, bq, Wk, bk, Wv, bv, Wm, bm, W1, b1, W2, b2):
    import ml_dtypes
    npF8 = mybir.dt.np(F8)
    npBF = ml_dtypes.bfloat16
    perm = np.array([4 * d + h for h in range(H) for d in range(DH)])
    f32 = lambda a: np.ascontiguousarray(a, dtype=np.float32)
    bf = lambda a: np.ascontiguousarray(np.asarray(a, np.float32), dtype=npBF)
    f8c = lambda a: np.ascontiguousarray(np.asarray(a, np.float32), dtype=npF8)

    b1p = np.asarray(b1, np.float64) + np.asarray(W1, np.float64)[:, D:] @ np.asarray(bm, np.float64)
    b1hi = np.asarray(b1p, np.float32).astype(npBF)
    b1lo = (np.asarray(b1p, np.float32) - b1hi.astype(np.float32)).astype(npBF)
    b2hi = np.asarray(b2, np.float32).astype(npBF)
    b2lo = (np.asarray(b2, np.float32) - b2hi.astype(np.float32)).astype(npBF)

    bvp = np.asarray(bv, np.float32)[perm]                    # (h, d) order
    bvb = np.tile(bvp.reshape(4, 1, DH), (1, 2, 1)).reshape(1, 8 * DH)
    bvb = np.ascontiguousarray(np.broadcast_to(bvb, (128, 8 * DH)), np.float32)

    shared = {
        "wq8": f8c(Wq[perm, :].T),
        "wk8": f8c(Wk[perm, :].T),
        "wv8": f8c(Wv[perm, :].T),
        "wmT": bf(Wm[:, perm].T),
        "w1xT": bf(W1.T[0:D, :]),
        "w1mT": bf(W1.T[D:2 * D, :]),
        "w2T": bf(W2.T),
        "bq": f32(bq[perm].reshape(2, 128).T),
        "bvb": bvb,
        "b1hl": np.ascontiguousarray(np.stack([b1hi, b1lo])),
        "b2hl": np.ascontiguousarray(np.stack([b2hi, b2lo])),
    }
    in_maps = []
    for core in range(NCORES):
        b, s = core // 4, core % 4
        m = dict(shared)
        xs = x[b][:, s * NS:(s + 1) * NS]
        m["xs8"] = f8c(xs)
        m["xsb"] = bf(xs)
        m["src8"] = f8c(source[b])
        in_maps.append(m)
    return in_maps


def run(inputs, **spmd_kwargs):
    """Build (cached), run on cores 0-7, return (full_output, BassKernelResults)."""
    nc = _build()
    in_maps = _prep_inputs(**inputs)
    res = bass_utils.run_bass_kernel_spmd(
        nc, in_maps, core_ids=list(range(NCORES)), **spmd_kwargs)
    full = np.empty((B, D, N), dtype=np.float32)
    for core in range(NCORES):
        b, s = core // 4, core % 4
        full[b][:, s * NS:(s + 1) * NS] = res.results[core]["out"]
    return full, res


def kernel(**inputs):
    full, _ = run(inputs)
    return full
